# revision 32
# baseline (speedup 1.0000x reference)
"""Trainium2 Bass kernel for EnhancedGraphTransformerLayer.

Layer: LN1 -> QKV proj -> per-node 8x8 head attention -> O proj -> residual
       -> LN2 -> FFN(512->2048->512, relu) -> residual.

Strategy (per NeuronCore, data-parallel over nodes, 8 cores):
- All big matmuls in bf16 on the PE (fp32 accumulate in PSUM).
- QKV projections are chunked over 4 node-groups (512-col matmuls,
  weights stationary) from a feature-transposed z chunk.
- Per-node 8-head attention uses "sub-group" packing: for each 16-node
  sub-group s, a (64, 128) slice layout q`T[d, h*16+j] lets one matmul
  compute all 128x128 head-pair scores; a block mask zeroes cross-node
  terms after exp, and an appended ones-column of V yields softmax
  denominators inside the AV matmul.
- The packing shuffles (q/k/v extraction, and the inverse extraction
  that feeds a K=128 O-projection) are merged into 2 DMAs per tensor
  and spread across the SP/Activation HWDGE queues + Pool SWDGE so they
  run concurrently with PE work instead of serializing on gpsimd.
- Attention is software-pipelined across groups (stage1: scores/AV and
  the repack DMA; stage2: O-proj + LN2) so the PE never waits on the
  inverse-extraction DMA latency.
- LayerNorm stats via bn_stats/bn_aggr in natural layout; gamma/beta
  folded into weights/biases on the host.
"""

import os

import numpy as np
import ml_dtypes
from contextlib import ExitStack

E = 512
H = 8
D = 64
F = 2048
EPS = 1e-5
N_NODES = 65536
N_CORES = 8
BF = ml_dtypes.bfloat16


def build_nc(npc, has_qkv_bias=False, has_bo=False, has_c2f=False,
             has_b2=False):
    import concourse.bass as bass
    import concourse.mybir as mybir

    f32 = mybir.dt.float32
    bf16 = mybir.dt.bfloat16

    nc = bass.Bass()
    ins = dict(
        x=nc.dram_tensor("x", (npc, E), f32, kind="ExternalInput").ap(),
        rwq=nc.dram_tensor("rwq", (E, E), bf16, kind="ExternalInput").ap(),
        rwk=nc.dram_tensor("rwk", (E, E), bf16, kind="ExternalInput").ap(),
        rwv=nc.dram_tensor("rwv", (E, E), bf16, kind="ExternalInput").ap(),
        rwo2=nc.dram_tensor("rwo2", (128, 4, E), bf16, kind="ExternalInput").ap(),
        rw1=nc.dram_tensor("rw1", (E, F), bf16, kind="ExternalInput").ap(),
        w2t=nc.dram_tensor("w2t", (F, E), bf16, kind="ExternalInput").ap(),
        mask=nc.dram_tensor("mask", (128, 128), bf16, kind="ExternalInput").ap(),
        c2q=nc.dram_tensor("c2q", (E,), f32, kind="ExternalInput").ap(),
        c2k=nc.dram_tensor("c2k", (E,), f32, kind="ExternalInput").ap(),
        c2v=nc.dram_tensor("c2v", (E,), f32, kind="ExternalInput").ap(),
        bo=nc.dram_tensor("bo", (E,), bf16, kind="ExternalInput").ap(),
        c2f=nc.dram_tensor("c2f", (F,), f32, kind="ExternalInput").ap(),
        b2=nc.dram_tensor("b2", (E,), f32, kind="ExternalInput").ap(),
    )
    out_ap = nc.dram_tensor("out", (npc, E), f32, kind="ExternalOutput").ap()
    build_body(nc, ins, out_ap, npc, has_qkv_bias=has_qkv_bias,
               has_bo=has_bo, has_c2f=has_c2f, has_b2=has_b2)
    return nc


def build_body(nc, ins, out_d, npc, has_qkv_bias=False, has_bo=False,
               has_c2f=False, has_b2=False):
    import concourse.bass as bass
    import concourse.mybir as mybir
    from concourse.tile import TileContext
    from concourse.masks import make_identity

    f32 = mybir.dt.float32
    bf16 = mybir.dt.bfloat16
    AL = mybir.AluOpType
    AF = mybir.ActivationFunctionType

    n_groups = npc // 128
    gpc = 4 if n_groups % 4 == 0 else 1  # groups per chunk
    n_chunks = n_groups // gpc

    x_d = ins["x"]
    rwq_d, rwk_d, rwv_d, rwo2_d = ins["rwq"], ins["rwk"], ins["rwv"], ins["rwo2"]
    rw1_d, w2t_d, mask_d = ins["rw1"], ins["w2t"], ins["mask"]
    c2q_d, c2k_d, c2v_d = ins["c2q"], ins["c2k"], ins["c2v"]
    bo_d, c2f_d, b2_d = ins["bo"], ins["c2f"], ins["b2"]

    with TileContext(nc) as tc, ExitStack() as ctx:
        wpool = ctx.enter_context(tc.tile_pool(name="w", bufs=1))
        pool = ctx.enter_context(tc.tile_pool(name="act", bufs=1))
        psum = ctx.enter_context(tc.tile_pool(name="ps", bufs=1, space="PSUM"))

        # ---- constants / weights ----
        rwq_sb = wpool.tile([128, 4, E], bf16, tag="rwq")
        rwk_sb = wpool.tile([128, 4, E], bf16, tag="rwk")
        rwv_sb = wpool.tile([128, 4, E], bf16, tag="rwv")
        rwo2_sb = wpool.tile([128, 4, E], bf16, tag="rwo2")
        nc.sync.dma_start(out=rwq_sb, in_=rwq_d.rearrange("(t p) e -> p t e", p=128))
        nc.sync.dma_start(out=rwk_sb, in_=rwk_d.rearrange("(t p) e -> p t e", p=128))
        nc.sync.dma_start(out=rwv_sb, in_=rwv_d.rearrange("(t p) e -> p t e", p=128))
        nc.sync.dma_start(out=rwo2_sb, in_=rwo2_d)
        rw1_sb = wpool.tile([128, 4, F], bf16, tag="rw1")
        nc.sync.dma_start(out=rw1_sb, in_=rw1_d.rearrange("(t p) f -> p t f", p=128))
        w2t_sb = wpool.tile([128, 16, E], bf16, tag="w2t")
        nc.sync.dma_start(out=w2t_sb, in_=w2t_d.rearrange("(t p) e -> p t e", p=128))
        mask_sb = wpool.tile([128, 128], bf16, tag="mask")
        nc.sync.dma_start(out=mask_sb, in_=mask_d)
        ident64 = wpool.tile([64, 64], bf16, tag="id64")
        make_identity(nc, ident64)
        ident128 = wpool.tile([128, 128], bf16, tag="id128")
        make_identity(nc, ident128)
        eps_sb = wpool.tile([128, 1], f32, tag="eps")
        nc.vector.memset(eps_sb, EPS)
        if has_qkv_bias:
            c2q_sb = wpool.tile([128, 4], f32, tag="c2q")
            c2k_sb = wpool.tile([128, 4], f32, tag="c2k")
            c2v_sb = wpool.tile([128, 4], f32, tag="c2v")
            nc.sync.dma_start(out=c2q_sb, in_=c2q_d.rearrange("(t p) -> p t", p=128))
            nc.sync.dma_start(out=c2k_sb, in_=c2k_d.rearrange("(t p) -> p t", p=128))
            nc.sync.dma_start(out=c2v_sb, in_=c2v_d.rearrange("(t p) -> p t", p=128))
        if has_bo:
            ones1_sb = wpool.tile([1, 128], bf16, tag="ones1")
            nc.vector.memset(ones1_sb, 1.0)
            bo_sb = wpool.tile([1, E], bf16, tag="bo")
            nc.sync.dma_start(out=bo_sb, in_=bo_d.rearrange("e -> 1 e"))
        if has_c2f:
            c2f_sb = wpool.tile([128, 16], f32, tag="c2f")
            nc.sync.dma_start(out=c2f_sb, in_=c2f_d.rearrange("(t p) -> p t", p=128))
        if has_b2:
            b2_sb = wpool.tile([128, 4], f32, tag="b2")
            nc.sync.dma_start(out=b2_sb, in_=b2_d.rearrange("(t p) -> p t", p=128))

        def bbox2(tile_ap):
            """2-element AP covering the tile's full byte range (fence)."""
            fs = 1
            for st, ct in tile_ap.ap[1:]:
                fs = max(fs, st * ct)
            return bass.AP(tensor=tile_ap.tensor, offset=tile_ap.offset,
                           ap=[tile_ap.ap[0], [fs - 1, 2]])

        def bcast8x64(small):
            """(128, 8) scalar AP broadcast to (128, 8, 64) via stride-0."""
            return bass.AP(tensor=small.tensor, offset=small.offset,
                           ap=[small.ap[0], [1, 8], [0, 64]])

        def headhalf_dst(tile_ap, par):
            """[64, 8(h), 128(n)] tile viewed as the parity-par head half:
            [64, tau(stride 256), n(128)] at column offset 128*par, i.e.
            head h=2*tau+par blocks."""
            return bass.AP(tensor=tile_ap.tensor,
                           offset=tile_ap.offset + 128 * par,
                           ap=[tile_ap.ap[0], [256, 4], [1, 128]])

        def sgrp(tile_ap, s):
            """[64, 8(h), 128(n)] tile sliced to sub-group s: [64 part,
            h(stride 128), j(16)] -> 128 slots streamed in (h, j) order."""
            return bass.AP(tensor=tile_ap.tensor,
                           offset=tile_ap.offset + 16 * s,
                           ap=[tile_ap.ap[0], [128, 8], [1, 16]])

        def ln_stats(x_sb, mvs, gi, tagp):
            """bn stats into mvs[:, gi, :] (mean, var)."""
            stat = pool.tile([128, 6], f32, tag=tagp + "stat", bufs=2, name=tagp + "stat")
            nc.vector.bn_stats(out=stat, in_=x_sb)
            nc.vector.bn_aggr(out=mvs[:, gi, :], in_=stat)

        def ln_sqrt_batch(mvs, tagp):
            """One Sqrt for the whole chunk: rs4[:, gi] = 1/sqrt(var_gi+eps).
            Batching keeps Exp<->Sqrt act-table switches to 2 per chunk."""
            rs4 = pool.tile([128, gpc], f32, tag=tagp + "rs4", bufs=2,
                            name=tagp + "rs4")
            nc.scalar.activation(out=rs4, in_=mvs[:, :, 1], func=AF.Sqrt,
                                 bias=eps_sb, scale=1.0)
            nc.vector.reciprocal(out=rs4, in_=rs4)
            return rs4

        def ln_norm(x_sb, mvs, rs4, gi, tagp):
            zb = pool.tile([128, E], bf16, tag=tagp + "zb", bufs=2, name=tagp + "zb")
            nc.vector.tensor_scalar(out=zb, in0=x_sb, scalar1=mvs[:, gi, 0:1],
                                    scalar2=rs4[:, gi:gi + 1], op0=AL.subtract,
                                    op1=AL.mult)
            return zb

        def prep(c):
            """Loads, LN1, z-transpose, chunked QKV, packing extractions."""
            P = {"x_tiles": [], "x2_tiles": []}
            zbT_chunk = pool.tile([128, 4, gpc, 128], bf16, tag="zchunk",
                                  bufs=2, name="zchunk")
            P["zbT_chunk"] = zbT_chunk
            mvs1 = pool.tile([128, gpc, 2], f32, tag="mvs1", bufs=2, name="mvs1")
            for gi in range(gpc):
                g = c * gpc + gi
                x_sb = pool.tile([128, E], f32, tag="x", bufs=gpc + 2, name="x_sb")
                nc.vector.memset(bbox2(x_sb), 0.0)
                nc.sync.dma_start(out=x_sb, in_=x_d[g * 128:(g + 1) * 128, :])
                P["x_tiles"].append(x_sb)
                ln_stats(x_sb, mvs1, gi, "ln1")
            rs41 = ln_sqrt_batch(mvs1, "ln1")
            for gi in range(gpc):
                zb = ln_norm(P["x_tiles"][gi], mvs1, rs41, gi, "ln1")
                zbT_ps = psum.tile([128, 4, 128], bf16, tag="A", bufs=4, name="zbT_ps")
                for tau in range(4):
                    nc.tensor.transpose(zbT_ps[:, tau, :],
                                        zb[:, 128 * tau:128 * (tau + 1)],
                                        ident128[:, :])
                nc.scalar.activation(out=zbT_chunk[:, :, gi, :], in_=zbT_ps,
                                     func=AF.Copy)

            # ---- QKV projections, chunked (512-col matmuls) ----
            qTb = pool.tile([128, 4, gpc, 128], bf16, tag="qTb", bufs=2, name="qTb")
            kTb = pool.tile([128, 4, gpc, 128], bf16, tag="kTb", bufs=2, name="kTb")
            vTb = pool.tile([128, 4, gpc, 128], bf16, tag="vTb", bufs=2, name="vTb")
            for tau in range(4):
                qT_ps = psum.tile([128, gpc * 128], f32, tag="A", bufs=4, name="qT_ps")
                kT_ps = psum.tile([128, gpc * 128], f32, tag="A", bufs=4, name="kT_ps")
                vT_ps = psum.tile([128, gpc * 128], f32, tag="A", bufs=4, name="vT_ps")
                for et in range(4):
                    st = et == 0
                    sp = et == 3
                    nc.tensor.matmul(qT_ps,
                                     rwq_sb[:, et, 128 * tau:128 * (tau + 1)],
                                     zbT_chunk[:, et, :, :], start=st, stop=sp)
                    nc.tensor.matmul(kT_ps,
                                     rwk_sb[:, et, 128 * tau:128 * (tau + 1)],
                                     zbT_chunk[:, et, :, :], start=st, stop=sp)
                    nc.tensor.matmul(vT_ps,
                                     rwv_sb[:, et, 128 * tau:128 * (tau + 1)],
                                     zbT_chunk[:, et, :, :], start=st, stop=sp)
                qv = qT_ps.rearrange("p (g n) -> p g n", g=gpc)
                kv = kT_ps.rearrange("p (g n) -> p g n", g=gpc)
                vv = vT_ps.rearrange("p (g n) -> p g n", g=gpc)
                if has_qkv_bias:
                    nc.scalar.activation(out=qTb[:, tau, :, :], in_=qv,
                                         func=AF.Identity, bias=c2q_sb[:, tau:tau + 1])
                    nc.scalar.activation(out=kTb[:, tau, :, :], in_=kv,
                                         func=AF.Identity, bias=c2k_sb[:, tau:tau + 1])
                    nc.vector.tensor_scalar_add(out=vTb[:, tau, :, :], in0=vv,
                                                scalar1=c2v_sb[:, tau:tau + 1])
                else:
                    nc.scalar.activation(out=qTb[:, tau, :, :], in_=qv, func=AF.Copy)
                    nc.scalar.activation(out=kTb[:, tau, :, :], in_=kv, func=AF.Copy)
                    nc.vector.tensor_copy(out=vTb[:, tau, :, :], in_=vv)

            # ---- packing extractions: per tensor per group, 2 partition-half
            #      DMAs into [d, h, n] layout (SP / Act / Pool queues), then an
            #      on-chip shuffle copy into the [d, s, h*16+j] matmul layout ----
            qxs, kxs, vxs = [], [], []
            for gi in range(gpc):
                q2 = pool.tile([64, 8, 128], bf16, tag="q2", bufs=2, name="q2")
                k2 = pool.tile([64, 8, 128], bf16, tag="k2", bufs=2, name="k2")
                v2 = pool.tile([64, 8, 128], bf16, tag="v2", bufs=2, name="v2")
                nc.scalar.activation(out=bbox2(q2), in_=bbox2(q2), func=AF.Copy)
                nc.vector.memset(bbox2(k2), 0.0)
                nc.vector.memset(bbox2(v2), 0.0)
                for par in range(2):
                    p0 = par * 64
                    nc.gpsimd.dma_start(out=headhalf_dst(v2, par),
                                        in_=vTb[p0:p0 + 64, :, gi, :],
                                        single_packet=True)
                    nc.sync.dma_start(out=headhalf_dst(q2, par),
                                      in_=qTb[p0:p0 + 64, :, gi, :])
                    nc.scalar.dma_start(out=headhalf_dst(k2, par),
                                        in_=kTb[p0:p0 + 64, :, gi, :])
                qx = pool.tile([64, 8, 128], bf16, tag="qx", bufs=2, name="qx")
                kx = pool.tile([64, 8, 128], bf16, tag="kx", bufs=2, name="kx")
                vx = pool.tile([64, 8, 128], bf16, tag="vx", bufs=2, name="vx")
                nc.scalar.copy(
                    out=vx.rearrange("d s (h j) -> d s h j", h=8),
                    in_=v2.rearrange("d h (s j) -> d s h j", s=8))
                nc.vector.tensor_copy(
                    out=qx.rearrange("d s (h j) -> d s h j", h=8),
                    in_=q2.rearrange("d h (s j) -> d s h j", s=8))
                nc.vector.tensor_copy(
                    out=kx.rearrange("d s (h j) -> d s h j", h=8),
                    in_=k2.rearrange("d h (s j) -> d s h j", s=8))
                qxs.append(qx)
                kxs.append(kx)
                vxs.append(vx)
            P["qxs"], P["kxs"], P["vxs"] = qxs, kxs, vxs
            return P

        def att_gen(c, P):
            """Attention for chunk c, software-pipelined over groups.
            Yields between pieces so FFN(c-1) matmuls can interleave."""
            qxs, kxs, vxs = P["qxs"], P["kxs"], P["vxs"]
            x_tiles, x2_tiles = P["x_tiles"], P["x2_tiles"]
            z2bT_chunk = pool.tile([128, 4, gpc, 128], bf16, tag="z2chunk",
                                   bufs=2, name="z2chunk")
            P["z2bT_chunk"] = z2bT_chunk

            def att_stage1(gi):
                qx, kx, vx = qxs[gi], kxs[gi], vxs[gi]
                # all scores first (exp/mask latency hides behind them)
                s2s = []
                for s0 in range(0, 8, 2):
                    s2_ps = psum.tile([128, 2, 128], f32, tag="A", bufs=4,
                                      name="s2_ps")
                    for si in range(2):
                        s = s0 + si
                        nc.tensor.matmul(s2_ps[:, si, :], kx[:, s, :], qx[:, s, :],
                                         start=True, stop=True)
                    e_sb = pool.tile([128, 2, 128], bf16, tag="esb", bufs=4,
                                     name="e_sb")
                    nc.scalar.activation(out=e_sb, in_=s2_ps, func=AF.Exp)
                    a_sb = pool.tile([128, 2, 128], bf16, tag="asb", bufs=4,
                                     name="a_sb")
                    nc.gpsimd.tensor_tensor(out=a_sb[:, 0, :], in0=e_sb[:, 0, :],
                                            in1=mask_sb, op=AL.mult)
                    nc.vector.tensor_tensor(out=a_sb[:, 1, :], in0=e_sb[:, 1, :],
                                            in1=mask_sb, op=AL.mult)
                    s2s.append(a_sb)
                yield

                # v packing: PE-transpose to [(g,j'), d] + ones column
                vp_ps = psum.tile([128, 8, 64], bf16, tag="A", bufs=4, name="vp_ps")
                for s in range(8):
                    nc.tensor.transpose(vp_ps[:, s, :], vx[:, s, :], ident64[:, :])
                vaug = pool.tile([128, 8, 66], bf16, tag="vaug", bufs=2, name="vaug")
                nc.vector.memset(vaug[:, :, 64:65], 1.0)
                nc.vector.tensor_copy(out=vaug[:, :, 0:64], in_=vp_ps)
                yield

                # AV (+denominator)
                # ([128, 8, 128] keeps each sub-group's slice PSUM-bank aligned)
                outS = psum.tile([128, 8, 128], f32, tag="B", bufs=2, name="outS")
                for s0 in range(0, 8, 2):
                    a_sb = s2s[s0 // 2]
                    for si in range(2):
                        s = s0 + si
                        nc.tensor.matmul(outS[:, s, 0:65], a_sb[:, si, :],
                                         vaug[:, s, 0:65], start=True, stop=True)
                yield

                # normalize + transpose to [d, h, n] layout
                recip = pool.tile([128, 8], f32, tag="recip", bufs=2, name="recip")
                nc.vector.reciprocal(out=recip, in_=outS[:, :, 64])
                ogb = pool.tile([128, 8, 64], bf16, tag="ogb", bufs=2, name="ogb")
                nc.vector.tensor_tensor(out=ogb, in0=outS[:, :, 0:64],
                                        in1=bcast8x64(recip), op=AL.mult)
                p_ps = psum.tile([64, 8, 128], bf16, tag="B", bufs=2, name="p_ps")
                for s in range(8):
                    nc.tensor.transpose(p_ps[:, s, :], ogb[:, s, :], ident128[:, :])
                # shuffle-copy to [d, h, n] while draining PSUM
                p_sb = pool.tile([64, 8, 128], bf16, tag="psb", bufs=2, name="p_sb")
                nc.vector.tensor_copy(
                    out=p_sb.rearrange("d h (s j) -> d s h j", s=8),
                    in_=p_ps.rearrange("d s (h j) -> d s h j", h=8))

                # inverse extraction: op2[(par*64+d), tau, n] <- p_sb[d, 2t+par, n]
                op2 = pool.tile([128, 4, 128], bf16, tag="op2", bufs=2, name="op2")
                nc.scalar.activation(out=bbox2(op2), in_=bbox2(op2), func=AF.Copy)
                for par in range(2):
                    nc.sync.dma_start(out=op2[64 * par:64 * (par + 1), :, :],
                                      in_=headhalf_dst(p_sb, par))
                yield
                return op2

            mvs2 = pool.tile([128, gpc, 2], f32, tag="mvs2", bufs=2, name="mvs2")

            def att_stage2a(gi, op2):
                # O projection, K=128 over 4 tau tiles -> natural [n, e]
                oproj_ps = psum.tile([128, E], f32, tag="A", bufs=4, name="oproj_ps")
                for tau in range(4):
                    nc.tensor.matmul(oproj_ps, op2[:, tau, :], rwo2_sb[:, tau, :],
                                     start=(tau == 0),
                                     stop=(tau == 3 and not has_bo))
                if has_bo:
                    nc.tensor.matmul(oproj_ps, ones1_sb, bo_sb,
                                     start=False, stop=True)

                # residual 1 + LN2 stats
                x2_sb = pool.tile([128, E], f32, tag="x2", bufs=gpc + 2,
                                  name="x2_sb")
                nc.vector.tensor_add(out=x2_sb, in0=x_tiles[gi], in1=oproj_ps)
                x2_tiles.append(x2_sb)
                ln_stats(x2_sb, mvs2, gi, "ln2")

            def att_stage2b(gi, rs42):
                z2b = ln_norm(x2_tiles[gi], mvs2, rs42, gi, "ln2")
                z2bT_ps = psum.tile([128, 4, 128], bf16, tag="A", bufs=4,
                                    name="z2bT_ps")
                for tau in range(4):
                    nc.tensor.transpose(z2bT_ps[:, tau, :],
                                        z2b[:, 128 * tau:128 * (tau + 1)],
                                        ident128[:, :])
                nc.scalar.activation(out=z2bT_chunk[:, :, gi, :], in_=z2bT_ps,
                                     func=AF.Copy)

            ops = [None] * gpc
            for gi in range(gpc):
                ops[gi] = yield from att_stage1(gi)
                if gi > 1:
                    att_stage2a(gi - 2, ops[gi - 2])
                    yield
            att_stage2a(gpc - 2, ops[gpc - 2])
            yield
            att_stage2a(gpc - 1, ops[gpc - 1])
            rs42 = ln_sqrt_batch(mvs2, "ln2")
            for gi in range(gpc):
                att_stage2b(gi, rs42)
                yield

        def ffn_gen(c, P):
            """FFN + output for chunk c. Yields between matmul groups."""
            z2bT_chunk = P["z2bT_chunk"]
            x2_tiles = P["x2_tiles"]
            rT_sb = pool.tile([128, 16, 128 * gpc], bf16, tag="rt", bufs=1, name="rT_sb")
            for ft in range(16):
                u1_ps = psum.tile([128, 128 * gpc], f32, tag="A", bufs=4, name="u1_ps")
                for et in range(4):
                    nc.tensor.matmul(u1_ps,
                                     rw1_sb[:, et, 128 * ft:128 * (ft + 1)],
                                     z2bT_chunk[:, et, :, :],
                                     start=(et == 0), stop=(et == 3))
                if has_c2f:
                    nc.vector.tensor_scalar(out=rT_sb[:, ft, :], in0=u1_ps,
                                            scalar1=c2f_sb[:, ft:ft + 1],
                                            scalar2=0.0, op0=AL.add, op1=AL.max)
                elif ft % 2 == 0:
                    nc.scalar.activation(out=rT_sb[:, ft, :], in_=u1_ps, func=AF.Relu)
                else:
                    nc.vector.tensor_scalar_max(out=rT_sb[:, ft, :], in0=u1_ps,
                                                scalar1=0.0)
                if ft % 2 == 1:
                    yield
            u2b_sb = pool.tile([128, 4, 128 * gpc], bf16, tag="u2b", bufs=2, name="u2b_sb")
            for et in range(4):
                u2_ps = psum.tile([128, 128 * gpc], f32, tag="A", bufs=4, name="u2_ps")
                for ft in range(16):
                    nc.tensor.matmul(u2_ps,
                                     w2t_sb[:, ft, 128 * et:128 * (et + 1)],
                                     rT_sb[:, ft, :],
                                     start=(ft == 0), stop=(ft == 15))
                if has_b2:
                    nc.vector.tensor_scalar_add(out=u2b_sb[:, et, :], in0=u2_ps,
                                                scalar1=b2_sb[:, et:et + 1])
                elif et % 2 == 0:
                    nc.scalar.activation(out=u2b_sb[:, et, :], in_=u2_ps, func=AF.Copy)
                else:
                    nc.vector.tensor_copy(out=u2b_sb[:, et, :], in_=u2_ps)
                yield
            u2nat = pool.tile([128, gpc, 4, 128], bf16, tag="u2nat", bufs=2, name="u2nat")
            for gi2 in range(gpc):
                u2n_ps = psum.tile([128, 4, 128], bf16, tag="A", bufs=4, name="u2n_ps")
                for et in range(4):
                    nc.tensor.transpose(u2n_ps[:, et, :],
                                        u2b_sb[:, et, 128 * gi2:128 * (gi2 + 1)],
                                        ident128[:, :])
                if gi2 % 2 == 0:
                    nc.vector.tensor_copy(out=u2nat[:, gi2, :, :], in_=u2n_ps)
                else:
                    nc.scalar.activation(out=u2nat[:, gi2, :, :], in_=u2n_ps,
                                         func=AF.Copy)
                yield
            for gi in range(gpc):
                g = c * gpc + gi
                out_sb = pool.tile([128, E], f32, tag="osb", bufs=3, name="out_sb")
                nc.vector.tensor_add(out=out_sb, in0=x2_tiles[gi],
                                     in1=u2nat[:, gi, :, :])
                nc.gpsimd.dma_start(out=out_d[g * 128:(g + 1) * 128, :], in_=out_sb)

        def drain_interleaved(g1, g2):
            """Round-robin two instruction-emitting generators."""
            gens = [g for g in (g1, g2) if g is not None]
            while gens:
                nxt = []
                for g in gens:
                    try:
                        next(g)
                        nxt.append(g)
                    except StopIteration:
                        pass
                gens = nxt

        # ---- main pipeline: ATT(c) interleaved with FFN(c-1), then PREP(c+1)
        P = [None] * n_chunks
        P[0] = prep(0)
        for c in range(n_chunks):
            drain_interleaved(att_gen(c, P[c]),
                              ffn_gen(c - 1, P[c - 1]) if c > 0 else None)
            if c + 1 < n_chunks:
                P[c + 1] = prep(c + 1)
        drain_interleaved(ffn_gen(n_chunks - 1, P[n_chunks - 1]), None)

    _fix_sync_waits(nc)


_DMA_LIKE = ("InstDMACopy", "InstDmaTransposeAnt", "InstDMATranspose",
             "InstKVWritebackAnt", "InstPagedWritebackAnt")


def _fix_sync_waits(nc):
    """walrus limits inline sync waits to 1 per instruction. Tile can
    emit more. Split the excess into
    standalone InstEventSemaphore wait-carriers inserted immediately before
    the overweight instruction on the same engine - semantically identical
    (the waits still execute right before the instruction, in order)."""
    import concourse.mybir as mybir
    n = 0
    for f in nc.m.functions:
        for blk in f.blocks:
            insts = blk.instructions
            out = []
            dirty = False
            for inst in insts:
                si = inst.sync_info
                waits = list(si.on_wait) if (si and si.on_wait) else []
                limit = 1
                if len(waits) > limit:
                    ups = list(si.on_update) if (si and si.on_update) else []
                    up_ids = {u.id for u in ups}
                    # keep own-queue credit waits inline (DMA flow control)
                    waits.sort(key=lambda w: 0 if w.id in up_ids else 1)
                    keep, move = waits[:limit], waits[limit:]
                    for w in move:
                        n += 1
                        car = mybir.InstEventSemaphore(
                            name="WSPLIT-%d" % n, ins=[], outs=[])
                        car.engine = inst.engine
                        car.sync_info = mybir.SyncInfo(on_wait=[w],
                                                       on_update=[])
                        out.append(car)
                    inst.sync_info = mybir.SyncInfo(on_wait=keep,
                                                   on_update=ups)
                    dirty = True
                out.append(inst)
            if dirty:
                blk.instructions = out
    return n


def _prep_weights(inputs):
    """Host-side weight folding. Returns dict of np arrays + flags."""
    f32 = np.float32
    g1 = np.asarray(inputs["g1"], f32)
    beta1 = np.asarray(inputs["beta1"], f32)
    g2 = np.asarray(inputs["g2"], f32)
    beta2 = np.asarray(inputs["beta2"], f32)
    Wq = np.asarray(inputs["Wq"], f32)
    Wk = np.asarray(inputs["Wk"], f32)
    Wv = np.asarray(inputs["Wv"], f32)
    Wo = np.asarray(inputs["Wo"], f32)
    W1 = np.asarray(inputs["W1"], f32)
    W2 = np.asarray(inputs["W2"], f32)
    scale = np.float32(1.0 / np.sqrt(D))

    rwq = (Wq.T * g1[:, None] * scale).astype(BF)
    rwk = (Wk.T * g1[:, None]).astype(BF)
    rwv = (Wv.T * g1[:, None]).astype(BF)
    # rwo2[par*64+d, tau, e] = Wo.T[(2*tau+par)*64+d, e]
    rwo2 = np.ascontiguousarray(
        Wo.T.reshape(4, 2, 64, E).transpose(1, 2, 0, 3).reshape(128, 4, E)
    ).astype(BF)
    rw1 = (W1.T * g2[:, None]).astype(BF)
    w2t = W2.T.astype(BF)

    c2q = ((Wq @ beta1 + np.asarray(inputs["bq"], f32)) * scale).astype(f32)
    c2k = (Wk @ beta1 + np.asarray(inputs["bk"], f32)).astype(f32)
    c2v = (Wv @ beta1 + np.asarray(inputs["bv"], f32)).astype(f32)
    bo = np.asarray(inputs["bo"], f32)
    c2f = (W1 @ beta2 + np.asarray(inputs["b1"], f32)).astype(f32)
    b2 = np.asarray(inputs["b2"], f32)

    mask = np.zeros((128, 128), f32)
    for i in range(16):
        for gg in range(8):
            for hh in range(8):
                mask[gg * 16 + i, hh * 16 + i] = 1.0

    return dict(
        rwq=rwq, rwk=rwk, rwv=rwv, rwo2=rwo2, rw1=rw1, w2t=w2t,
        mask=mask.astype(BF),
        c2q=c2q, c2k=c2k, c2v=c2v, bo=bo.astype(BF), c2f=c2f, b2=b2,
        has_qkv_bias=bool(np.any(c2q) or np.any(c2k) or np.any(c2v)),
        has_bo=bool(np.any(bo)), has_c2f=bool(np.any(c2f)),
        has_b2=bool(np.any(b2)),
    )


def kernel(**inputs):
    from concourse.bass_utils import run_bass_kernel_spmd

    x = np.asarray(inputs["x"], np.float32)
    n = x.shape[0]
    npc = n // N_CORES
    w = _prep_weights(inputs)

    nc = build_nc(npc, has_qkv_bias=w["has_qkv_bias"], has_bo=w["has_bo"],
                  has_c2f=w["has_c2f"], has_b2=w["has_b2"])

    shared = dict(rwq=w["rwq"], rwk=w["rwk"], rwv=w["rwv"], rwo2=w["rwo2"],
                  rw1=w["rw1"], w2t=w["w2t"], mask=w["mask"],
                  c2q=w["c2q"], c2k=w["c2k"], c2v=w["c2v"], bo=w["bo"],
                  c2f=w["c2f"], b2=w["b2"])
    in_maps = []
    for core in range(N_CORES):
        m = dict(shared)
        m["x"] = np.ascontiguousarray(x[core * npc:(core + 1) * npc])
        in_maps.append(m)

    res = run_bass_kernel_spmd(nc, in_maps, list(range(N_CORES)))
    out = np.concatenate([np.asarray(res.results[c]["out"])
                          for c in range(N_CORES)], axis=0)
    return out.astype(np.float32)


# revision 33
# speedup vs baseline: 1.1651x; 1.1651x over previous
"""Trainium2 Bass kernel for EnhancedGraphTransformerLayer.

Layer: LN1 -> QKV proj -> per-node 8x8 head attention -> O proj -> residual
       -> LN2 -> FFN(512->2048->512, relu) -> residual.

Strategy (per NeuronCore, data-parallel over nodes, 8 cores):
- All big matmuls in bf16 on the PE (fp32 accumulate in PSUM).
- QKV projections are chunked over 4 node-groups (512-col matmuls,
  weights stationary) from a feature-transposed z chunk.
- Per-node 8-head attention uses "sub-group" packing: for each 16-node
  sub-group s, a (64, 128) slice layout q`T[d, h*16+j] lets one matmul
  compute all 128x128 head-pair scores; a block mask zeroes cross-node
  terms after exp, and an appended ones-column of V yields softmax
  denominators inside the AV matmul.
- The packing shuffles (q/k/v extraction, and the inverse extraction
  that feeds a K=128 O-projection) are merged into 2 DMAs per tensor
  and spread across the SP/Activation HWDGE queues + Pool SWDGE so they
  run concurrently with PE work instead of serializing on gpsimd.
- Attention is software-pipelined across groups (stage1: scores/AV and
  the repack DMA; stage2: O-proj + LN2) so the PE never waits on the
  inverse-extraction DMA latency.
- LayerNorm stats via bn_stats/bn_aggr in natural layout; gamma/beta
  folded into weights/biases on the host.
"""

import os

import numpy as np
import ml_dtypes
from contextlib import ExitStack

E = 512
H = 8
D = 64
F = 2048
EPS = 1e-5
FP8_S1 = 64.0
FP8_S2 = 64.0
N_NODES = 65536
N_CORES = 8
BF = ml_dtypes.bfloat16
FP8 = ml_dtypes.float8_e4m3


def build_nc(npc, has_qkv_bias=False, has_bo=False, has_c2f=False,
             has_b2=False):
    import concourse.bass as bass
    import concourse.mybir as mybir

    f32 = mybir.dt.float32
    bf16 = mybir.dt.bfloat16
    fp8 = mybir.dt.float8e4

    nc = bass.Bass()
    ins = dict(
        x=nc.dram_tensor("x", (npc, E), f32, kind="ExternalInput").ap(),
        rwq=nc.dram_tensor("rwq", (E, E), bf16, kind="ExternalInput").ap(),
        rwk=nc.dram_tensor("rwk", (E, E), bf16, kind="ExternalInput").ap(),
        rwv=nc.dram_tensor("rwv", (E, E), bf16, kind="ExternalInput").ap(),
        rwo2=nc.dram_tensor("rwo2", (128, 4, E), bf16, kind="ExternalInput").ap(),
        rw1=nc.dram_tensor("rw1", (E, F), fp8, kind="ExternalInput").ap(),
        w2t=nc.dram_tensor("w2t", (F, E), fp8, kind="ExternalInput").ap(),
        mask=nc.dram_tensor("mask", (128, 128), bf16, kind="ExternalInput").ap(),
        c2q=nc.dram_tensor("c2q", (E,), f32, kind="ExternalInput").ap(),
        c2k=nc.dram_tensor("c2k", (E,), f32, kind="ExternalInput").ap(),
        c2v=nc.dram_tensor("c2v", (E,), f32, kind="ExternalInput").ap(),
        bo=nc.dram_tensor("bo", (E,), bf16, kind="ExternalInput").ap(),
        c2f=nc.dram_tensor("c2f", (F,), f32, kind="ExternalInput").ap(),
        b2=nc.dram_tensor("b2", (E,), f32, kind="ExternalInput").ap(),
    )
    out_ap = nc.dram_tensor("out", (npc, E), f32, kind="ExternalOutput").ap()
    build_body(nc, ins, out_ap, npc, has_qkv_bias=has_qkv_bias,
               has_bo=has_bo, has_c2f=has_c2f, has_b2=has_b2)
    return nc


def build_body(nc, ins, out_d, npc, has_qkv_bias=False, has_bo=False,
               has_c2f=False, has_b2=False):
    import concourse.bass as bass
    import concourse.mybir as mybir
    from concourse.tile import TileContext
    from concourse.masks import make_identity

    f32 = mybir.dt.float32
    bf16 = mybir.dt.bfloat16
    fp8 = mybir.dt.float8e4
    PM = mybir.MatmulPerfMode
    AL = mybir.AluOpType
    AF = mybir.ActivationFunctionType

    n_groups = npc // 128
    gpc = 4 if n_groups % 4 == 0 else 1  # groups per chunk
    n_chunks = n_groups // gpc

    x_d = ins["x"]
    rwq_d, rwk_d, rwv_d, rwo2_d = ins["rwq"], ins["rwk"], ins["rwv"], ins["rwo2"]
    rw1_d, w2t_d, mask_d = ins["rw1"], ins["w2t"], ins["mask"]
    c2q_d, c2k_d, c2v_d = ins["c2q"], ins["c2k"], ins["c2v"]
    bo_d, c2f_d, b2_d = ins["bo"], ins["c2f"], ins["b2"]

    with TileContext(nc) as tc, ExitStack() as ctx:
        wpool = ctx.enter_context(tc.tile_pool(name="w", bufs=1))
        pool = ctx.enter_context(tc.tile_pool(name="act", bufs=1))
        psum = ctx.enter_context(tc.tile_pool(name="ps", bufs=1, space="PSUM"))

        # ---- constants / weights ----
        rwq_sb = wpool.tile([128, 4, E], bf16, tag="rwq")
        rwk_sb = wpool.tile([128, 4, E], bf16, tag="rwk")
        rwv_sb = wpool.tile([128, 4, E], bf16, tag="rwv")
        rwo2_sb = wpool.tile([128, 4, E], bf16, tag="rwo2")
        nc.sync.dma_start(out=rwq_sb, in_=rwq_d.rearrange("(t p) e -> p t e", p=128))
        nc.sync.dma_start(out=rwk_sb, in_=rwk_d.rearrange("(t p) e -> p t e", p=128))
        nc.sync.dma_start(out=rwv_sb, in_=rwv_d.rearrange("(t p) e -> p t e", p=128))
        nc.sync.dma_start(out=rwo2_sb, in_=rwo2_d)
        rw1_sb = wpool.tile([128, 4, F], fp8, tag="rw1")
        nc.sync.dma_start(out=rw1_sb, in_=rw1_d.rearrange("(t p) f -> p t f", p=128))
        w2t_sb = wpool.tile([128, 16, E], fp8, tag="w2t")
        nc.sync.dma_start(out=w2t_sb, in_=w2t_d.rearrange("(t p) e -> p t e", p=128))
        mask_sb = wpool.tile([128, 128], bf16, tag="mask")
        nc.sync.dma_start(out=mask_sb, in_=mask_d)
        ident64 = wpool.tile([64, 64], bf16, tag="id64")
        make_identity(nc, ident64)
        ident128 = wpool.tile([128, 128], bf16, tag="id128")
        make_identity(nc, ident128)
        eps_sb = wpool.tile([128, 1], f32, tag="eps")
        nc.vector.memset(eps_sb, EPS)
        if has_qkv_bias:
            c2q_sb = wpool.tile([128, 4], f32, tag="c2q")
            c2k_sb = wpool.tile([128, 4], f32, tag="c2k")
            c2v_sb = wpool.tile([128, 4], f32, tag="c2v")
            nc.sync.dma_start(out=c2q_sb, in_=c2q_d.rearrange("(t p) -> p t", p=128))
            nc.sync.dma_start(out=c2k_sb, in_=c2k_d.rearrange("(t p) -> p t", p=128))
            nc.sync.dma_start(out=c2v_sb, in_=c2v_d.rearrange("(t p) -> p t", p=128))
        if has_bo:
            ones1_sb = wpool.tile([1, 128], bf16, tag="ones1")
            nc.vector.memset(ones1_sb, 1.0)
            bo_sb = wpool.tile([1, E], bf16, tag="bo")
            nc.sync.dma_start(out=bo_sb, in_=bo_d.rearrange("e -> 1 e"))
        if has_c2f:
            c2f_sb = wpool.tile([128, 16], f32, tag="c2f")
            nc.sync.dma_start(out=c2f_sb, in_=c2f_d.rearrange("(t p) -> p t", p=128))
        if has_b2:
            b2_sb = wpool.tile([128, 4], f32, tag="b2")
            nc.sync.dma_start(out=b2_sb, in_=b2_d.rearrange("(t p) -> p t", p=128))

        def bbox2(tile_ap):
            """2-element AP covering the tile's full byte range (fence)."""
            fs = 1
            for st, ct in tile_ap.ap[1:]:
                fs = max(fs, st * ct)
            return bass.AP(tensor=tile_ap.tensor, offset=tile_ap.offset,
                           ap=[tile_ap.ap[0], [fs - 1, 2]])

        def bcast8x64(small):
            """(128, 8) scalar AP broadcast to (128, 8, 64) via stride-0."""
            return bass.AP(tensor=small.tensor, offset=small.offset,
                           ap=[small.ap[0], [1, 8], [0, 64]])

        def headhalf_dst(tile_ap, par):
            """[64, 8(h), 128(n)] tile viewed as the parity-par head half:
            [64, tau(stride 256), n(128)] at column offset 128*par, i.e.
            head h=2*tau+par blocks."""
            return bass.AP(tensor=tile_ap.tensor,
                           offset=tile_ap.offset + 128 * par,
                           ap=[tile_ap.ap[0], [256, 4], [1, 128]])

        def sgrp(tile_ap, s):
            """[64, 8(h), 128(n)] tile sliced to sub-group s: [64 part,
            h(stride 128), j(16)] -> 128 slots streamed in (h, j) order."""
            return bass.AP(tensor=tile_ap.tensor,
                           offset=tile_ap.offset + 16 * s,
                           ap=[tile_ap.ap[0], [128, 8], [1, 16]])

        def ln_stats(x_sb, mvs, gi, tagp):
            """bn stats into mvs[:, gi, :] (mean, var)."""
            stat = pool.tile([128, 6], f32, tag=tagp + "stat", bufs=2, name=tagp + "stat")
            nc.vector.bn_stats(out=stat, in_=x_sb)
            nc.vector.bn_aggr(out=mvs[:, gi, :], in_=stat)

        def ln_sqrt_batch(mvs, tagp):
            """One Sqrt for the whole chunk: rs4[:, gi] = 1/sqrt(var_gi+eps).
            Batching keeps Exp<->Sqrt act-table switches to 2 per chunk."""
            rs4 = pool.tile([128, gpc], f32, tag=tagp + "rs4", bufs=2,
                            name=tagp + "rs4")
            nc.scalar.activation(out=rs4, in_=mvs[:, :, 1], func=AF.Sqrt,
                                 bias=eps_sb, scale=1.0)
            nc.vector.reciprocal(out=rs4, in_=rs4)
            return rs4

        def ln_norm(x_sb, mvs, rs4, gi, tagp):
            zb = pool.tile([128, E], bf16, tag=tagp + "zb", bufs=2, name=tagp + "zb")
            nc.vector.tensor_scalar(out=zb, in0=x_sb, scalar1=mvs[:, gi, 0:1],
                                    scalar2=rs4[:, gi:gi + 1], op0=AL.subtract,
                                    op1=AL.mult)
            return zb

        def prep(c):
            """Loads, LN1, z-transpose, chunked QKV, packing extractions."""
            P = {"x_tiles": [], "x2_tiles": []}
            zbT_chunk = pool.tile([128, 4, gpc, 128], bf16, tag="zchunk",
                                  bufs=2, name="zchunk")
            P["zbT_chunk"] = zbT_chunk
            mvs1 = pool.tile([128, gpc, 2], f32, tag="mvs1", bufs=2, name="mvs1")
            for gi in range(gpc):
                g = c * gpc + gi
                x_sb = pool.tile([128, E], f32, tag="x", bufs=gpc + 2, name="x_sb")
                nc.vector.memset(bbox2(x_sb), 0.0)
                nc.sync.dma_start(out=x_sb, in_=x_d[g * 128:(g + 1) * 128, :])
                P["x_tiles"].append(x_sb)
                ln_stats(x_sb, mvs1, gi, "ln1")
            rs41 = ln_sqrt_batch(mvs1, "ln1")
            for gi in range(gpc):
                zb = ln_norm(P["x_tiles"][gi], mvs1, rs41, gi, "ln1")
                zbT_ps = psum.tile([128, 4, 128], bf16, tag="A", bufs=4, name="zbT_ps")
                for tau in range(4):
                    nc.tensor.transpose(zbT_ps[:, tau, :],
                                        zb[:, 128 * tau:128 * (tau + 1)],
                                        ident128[:, :])
                nc.scalar.activation(out=zbT_chunk[:, :, gi, :], in_=zbT_ps,
                                     func=AF.Copy)

            # ---- QKV projections, chunked (512-col matmuls) ----
            qTb = pool.tile([128, 4, gpc, 128], bf16, tag="qTb", bufs=2, name="qTb")
            kTb = pool.tile([128, 4, gpc, 128], bf16, tag="kTb", bufs=2, name="kTb")
            vTb = pool.tile([128, 4, gpc, 128], bf16, tag="vTb", bufs=2, name="vTb")
            for tau in range(4):
                qT_ps = psum.tile([128, gpc * 128], f32, tag="A", bufs=4, name="qT_ps")
                kT_ps = psum.tile([128, gpc * 128], f32, tag="A", bufs=4, name="kT_ps")
                vT_ps = psum.tile([128, gpc * 128], f32, tag="A", bufs=4, name="vT_ps")
                for et in range(4):
                    st = et == 0
                    sp = et == 3
                    nc.tensor.matmul(qT_ps,
                                     rwq_sb[:, et, 128 * tau:128 * (tau + 1)],
                                     zbT_chunk[:, et, :, :], start=st, stop=sp)
                    nc.tensor.matmul(kT_ps,
                                     rwk_sb[:, et, 128 * tau:128 * (tau + 1)],
                                     zbT_chunk[:, et, :, :], start=st, stop=sp)
                    nc.tensor.matmul(vT_ps,
                                     rwv_sb[:, et, 128 * tau:128 * (tau + 1)],
                                     zbT_chunk[:, et, :, :], start=st, stop=sp)
                qv = qT_ps.rearrange("p (g n) -> p g n", g=gpc)
                kv = kT_ps.rearrange("p (g n) -> p g n", g=gpc)
                vv = vT_ps.rearrange("p (g n) -> p g n", g=gpc)
                if has_qkv_bias:
                    nc.scalar.activation(out=qTb[:, tau, :, :], in_=qv,
                                         func=AF.Identity, bias=c2q_sb[:, tau:tau + 1])
                    nc.scalar.activation(out=kTb[:, tau, :, :], in_=kv,
                                         func=AF.Identity, bias=c2k_sb[:, tau:tau + 1])
                    nc.vector.tensor_scalar_add(out=vTb[:, tau, :, :], in0=vv,
                                                scalar1=c2v_sb[:, tau:tau + 1])
                else:
                    nc.scalar.activation(out=qTb[:, tau, :, :], in_=qv, func=AF.Copy)
                    nc.scalar.activation(out=kTb[:, tau, :, :], in_=kv, func=AF.Copy)
                    nc.vector.tensor_copy(out=vTb[:, tau, :, :], in_=vv)

            # ---- packing extractions: per tensor per group, 2 partition-half
            #      DMAs into [d, h, n] layout (SP / Act / Pool queues), then an
            #      on-chip shuffle copy into the [d, s, h*16+j] matmul layout ----
            qxs, kxs, vxs = [], [], []
            for gi in range(gpc):
                q2 = pool.tile([64, 8, 128], bf16, tag="q2", bufs=2, name="q2")
                k2 = pool.tile([64, 8, 128], bf16, tag="k2", bufs=2, name="k2")
                v2 = pool.tile([64, 8, 128], bf16, tag="v2", bufs=2, name="v2")
                nc.scalar.activation(out=bbox2(q2), in_=bbox2(q2), func=AF.Copy)
                nc.vector.memset(bbox2(k2), 0.0)
                nc.vector.memset(bbox2(v2), 0.0)
                for par in range(2):
                    p0 = par * 64
                    nc.gpsimd.dma_start(out=headhalf_dst(v2, par),
                                        in_=vTb[p0:p0 + 64, :, gi, :],
                                        single_packet=True)
                    nc.sync.dma_start(out=headhalf_dst(q2, par),
                                      in_=qTb[p0:p0 + 64, :, gi, :])
                    nc.scalar.dma_start(out=headhalf_dst(k2, par),
                                        in_=kTb[p0:p0 + 64, :, gi, :])
                qx = pool.tile([64, 8, 128], bf16, tag="qx", bufs=2, name="qx")
                kx = pool.tile([64, 8, 128], bf16, tag="kx", bufs=2, name="kx")
                vx = pool.tile([64, 8, 128], bf16, tag="vx", bufs=2, name="vx")
                nc.scalar.copy(
                    out=vx.rearrange("d s (h j) -> d s h j", h=8),
                    in_=v2.rearrange("d h (s j) -> d s h j", s=8))
                nc.vector.tensor_copy(
                    out=qx.rearrange("d s (h j) -> d s h j", h=8),
                    in_=q2.rearrange("d h (s j) -> d s h j", s=8))
                nc.vector.tensor_copy(
                    out=kx.rearrange("d s (h j) -> d s h j", h=8),
                    in_=k2.rearrange("d h (s j) -> d s h j", s=8))
                qxs.append(qx)
                kxs.append(kx)
                vxs.append(vx)
            P["qxs"], P["kxs"], P["vxs"] = qxs, kxs, vxs
            return P

        def att_gen(c, P):
            """Attention for chunk c, software-pipelined over groups.
            Yields between pieces so FFN(c-1) matmuls can interleave."""
            qxs, kxs, vxs = P["qxs"], P["kxs"], P["vxs"]
            x_tiles, x2_tiles = P["x_tiles"], P["x2_tiles"]
            z2bT_chunk = pool.tile([128, 4, gpc, 128], fp8, tag="z2chunk",
                                   bufs=2, name="z2chunk")
            P["z2bT_chunk"] = z2bT_chunk

            def att_stage1(gi):
                qx, kx, vx = qxs[gi], kxs[gi], vxs[gi]
                # all scores first (exp/mask latency hides behind them)
                s2s = []
                for s0 in range(0, 8, 2):
                    s2_ps = psum.tile([128, 2, 128], f32, tag="A", bufs=4,
                                      name="s2_ps")
                    for si in range(2):
                        s = s0 + si
                        nc.tensor.matmul(s2_ps[:, si, :], kx[:, s, :], qx[:, s, :],
                                         start=True, stop=True)
                    e_sb = pool.tile([128, 2, 128], bf16, tag="esb", bufs=4,
                                     name="e_sb")
                    nc.scalar.activation(out=e_sb, in_=s2_ps, func=AF.Exp)
                    a_sb = pool.tile([128, 2, 128], bf16, tag="asb", bufs=4,
                                     name="a_sb")
                    nc.gpsimd.tensor_tensor(out=a_sb[:, 0, :], in0=e_sb[:, 0, :],
                                            in1=mask_sb, op=AL.mult)
                    nc.vector.tensor_tensor(out=a_sb[:, 1, :], in0=e_sb[:, 1, :],
                                            in1=mask_sb, op=AL.mult)
                    s2s.append(a_sb)
                yield

                # v packing: PE-transpose to [(g,j'), d] + ones column
                vp_ps = psum.tile([128, 8, 64], bf16, tag="A", bufs=4, name="vp_ps")
                for s in range(8):
                    nc.tensor.transpose(vp_ps[:, s, :], vx[:, s, :], ident64[:, :])
                vaug = pool.tile([128, 8, 66], bf16, tag="vaug", bufs=2, name="vaug")
                nc.vector.memset(vaug[:, :, 64:65], 1.0)
                nc.vector.tensor_copy(out=vaug[:, :, 0:64], in_=vp_ps)
                yield

                # AV (+denominator)
                # ([128, 8, 128] keeps each sub-group's slice PSUM-bank aligned)
                outS = psum.tile([128, 8, 128], f32, tag="B", bufs=2, name="outS")
                for s0 in range(0, 8, 2):
                    a_sb = s2s[s0 // 2]
                    for si in range(2):
                        s = s0 + si
                        nc.tensor.matmul(outS[:, s, 0:65], a_sb[:, si, :],
                                         vaug[:, s, 0:65], start=True, stop=True)
                yield

                # normalize + transpose to [d, h, n] layout
                recip = pool.tile([128, 8], f32, tag="recip", bufs=2, name="recip")
                nc.vector.reciprocal(out=recip, in_=outS[:, :, 64])
                ogb = pool.tile([128, 8, 64], bf16, tag="ogb", bufs=2, name="ogb")
                nc.vector.tensor_tensor(out=ogb, in0=outS[:, :, 0:64],
                                        in1=bcast8x64(recip), op=AL.mult)
                p_ps = psum.tile([64, 8, 128], bf16, tag="B", bufs=2, name="p_ps")
                for s in range(8):
                    nc.tensor.transpose(p_ps[:, s, :], ogb[:, s, :], ident128[:, :])
                # shuffle-copy to [d, h, n] while draining PSUM
                p_sb = pool.tile([64, 8, 128], bf16, tag="psb", bufs=2, name="p_sb")
                nc.vector.tensor_copy(
                    out=p_sb.rearrange("d h (s j) -> d s h j", s=8),
                    in_=p_ps.rearrange("d s (h j) -> d s h j", h=8))

                # inverse extraction: op2[(par*64+d), tau, n] <- p_sb[d, 2t+par, n]
                op2 = pool.tile([128, 4, 128], bf16, tag="op2", bufs=2, name="op2")
                nc.scalar.activation(out=bbox2(op2), in_=bbox2(op2), func=AF.Copy)
                for par in range(2):
                    nc.sync.dma_start(out=op2[64 * par:64 * (par + 1), :, :],
                                      in_=headhalf_dst(p_sb, par))
                yield
                return op2

            mvs2 = pool.tile([128, gpc, 2], f32, tag="mvs2", bufs=2, name="mvs2")

            def att_stage2a(gi, op2):
                # O projection, K=128 over 4 tau tiles -> natural [n, e]
                oproj_ps = psum.tile([128, E], f32, tag="A", bufs=4, name="oproj_ps")
                for tau in range(4):
                    nc.tensor.matmul(oproj_ps, op2[:, tau, :], rwo2_sb[:, tau, :],
                                     start=(tau == 0),
                                     stop=(tau == 3 and not has_bo))
                if has_bo:
                    nc.tensor.matmul(oproj_ps, ones1_sb, bo_sb,
                                     start=False, stop=True)

                # residual 1 + LN2 stats
                x2_sb = pool.tile([128, E], f32, tag="x2", bufs=gpc + 2,
                                  name="x2_sb")
                nc.vector.tensor_add(out=x2_sb, in0=x_tiles[gi], in1=oproj_ps)
                x2_tiles.append(x2_sb)
                ln_stats(x2_sb, mvs2, gi, "ln2")

            def att_stage2b(gi, rs42):
                z2b = ln_norm(x2_tiles[gi], mvs2, rs42, gi, "ln2")
                z2bT_ps = psum.tile([128, 4, 128], bf16, tag="A", bufs=4,
                                    name="z2bT_ps")
                for tau in range(4):
                    nc.tensor.transpose(z2bT_ps[:, tau, :],
                                        z2b[:, 128 * tau:128 * (tau + 1)],
                                        ident128[:, :])
                nc.scalar.activation(out=z2bT_chunk[:, :, gi, :], in_=z2bT_ps,
                                     func=AF.Copy)

            ops = [None] * gpc
            for gi in range(gpc):
                ops[gi] = yield from att_stage1(gi)
                if gi > 1:
                    att_stage2a(gi - 2, ops[gi - 2])
                    yield
            att_stage2a(gpc - 2, ops[gpc - 2])
            yield
            att_stage2a(gpc - 1, ops[gpc - 1])
            rs42 = ln_sqrt_batch(mvs2, "ln2")
            for gi in range(gpc):
                att_stage2b(gi, rs42)
                yield

        def ffn_gen(c, P):
            """FFN + output for chunk c. Yields between matmul groups."""
            z2bT_chunk = P["z2bT_chunk"]
            x2_tiles = P["x2_tiles"]
            rT_sb = pool.tile([128, 16, 128 * gpc], fp8, tag="rt", bufs=1, name="rT_sb")
            for ft in range(16):
                u1_ps = psum.tile([128, 128 * gpc], f32, tag="A", bufs=4, name="u1_ps")
                for u in range(2):
                    nc.tensor.matmul(u1_ps,
                                     rw1_sb[:, 2 * u:2 * u + 2, 128 * ft:128 * (ft + 1)],
                                     z2bT_chunk[:, 2 * u:2 * u + 2, :, :],
                                     start=(u == 0), stop=(u == 1),
                                     perf_mode=PM.DoubleRow)
                if has_c2f:
                    nc.scalar.activation(out=rT_sb[:, ft, :], in_=u1_ps,
                                         func=AF.Relu, bias=c2f_sb[:, ft:ft + 1],
                                         scale=1.0 / FP8_S1)
                elif ft % 2 == 0:
                    nc.scalar.activation(out=rT_sb[:, ft, :], in_=u1_ps,
                                         func=AF.Relu, scale=1.0 / FP8_S1)
                else:
                    nc.vector.tensor_scalar(out=rT_sb[:, ft, :], in0=u1_ps,
                                            scalar1=1.0 / FP8_S1, scalar2=0.0,
                                            op0=AL.mult, op1=AL.max)
                if ft % 2 == 1:
                    yield
            u2b_sb = pool.tile([128, 4, 128 * gpc], bf16, tag="u2b", bufs=2, name="u2b_sb")
            for et in range(4):
                u2_ps = psum.tile([128, 128 * gpc], f32, tag="A", bufs=4, name="u2_ps")
                for u in range(8):
                    nc.tensor.matmul(u2_ps,
                                     w2t_sb[:, 2 * u:2 * u + 2, 128 * et:128 * (et + 1)],
                                     rT_sb[:, 2 * u:2 * u + 2, :],
                                     start=(u == 0), stop=(u == 7),
                                     perf_mode=PM.DoubleRow)
                if has_b2:
                    nc.vector.tensor_scalar(out=u2b_sb[:, et, :], in0=u2_ps,
                                            scalar1=1.0 / FP8_S2,
                                            scalar2=b2_sb[:, et:et + 1],
                                            op0=AL.mult, op1=AL.add)
                elif et % 2 == 0:
                    nc.scalar.activation(out=u2b_sb[:, et, :], in_=u2_ps,
                                         func=AF.Copy, scale=1.0 / FP8_S2)
                else:
                    nc.vector.tensor_scalar(out=u2b_sb[:, et, :], in0=u2_ps,
                                            scalar1=1.0 / FP8_S2, scalar2=0.0,
                                            op0=AL.mult, op1=AL.add)
                yield
            u2nat = pool.tile([128, gpc, 4, 128], bf16, tag="u2nat", bufs=2, name="u2nat")
            for gi2 in range(gpc):
                u2n_ps = psum.tile([128, 4, 128], bf16, tag="A", bufs=4, name="u2n_ps")
                for et in range(4):
                    nc.tensor.transpose(u2n_ps[:, et, :],
                                        u2b_sb[:, et, 128 * gi2:128 * (gi2 + 1)],
                                        ident128[:, :])
                if gi2 % 2 == 0:
                    nc.vector.tensor_copy(out=u2nat[:, gi2, :, :], in_=u2n_ps)
                else:
                    nc.scalar.activation(out=u2nat[:, gi2, :, :], in_=u2n_ps,
                                         func=AF.Copy)
                yield
            for gi in range(gpc):
                g = c * gpc + gi
                out_sb = pool.tile([128, E], f32, tag="osb", bufs=3, name="out_sb")
                nc.vector.tensor_add(out=out_sb, in0=x2_tiles[gi],
                                     in1=u2nat[:, gi, :, :])
                nc.gpsimd.dma_start(out=out_d[g * 128:(g + 1) * 128, :], in_=out_sb)

        def drain_interleaved(g1, g2):
            """Round-robin two instruction-emitting generators."""
            gens = [g for g in (g1, g2) if g is not None]
            while gens:
                nxt = []
                for g in gens:
                    try:
                        next(g)
                        nxt.append(g)
                    except StopIteration:
                        pass
                gens = nxt

        # ---- main pipeline: ATT(c) interleaved with FFN(c-1), then PREP(c+1)
        P = [None] * n_chunks
        P[0] = prep(0)
        for c in range(n_chunks):
            drain_interleaved(att_gen(c, P[c]),
                              ffn_gen(c - 1, P[c - 1]) if c > 0 else None)
            if c + 1 < n_chunks:
                P[c + 1] = prep(c + 1)
        drain_interleaved(ffn_gen(n_chunks - 1, P[n_chunks - 1]), None)

    _fix_sync_waits(nc)


_DMA_LIKE = ("InstDMACopy", "InstDmaTransposeAnt", "InstDMATranspose",
             "InstKVWritebackAnt", "InstPagedWritebackAnt")


def _fix_sync_waits(nc):
    """walrus limits inline sync waits to 1 per instruction. Tile can
    emit more. Split the excess into
    standalone InstEventSemaphore wait-carriers inserted immediately before
    the overweight instruction on the same engine - semantically identical
    (the waits still execute right before the instruction, in order)."""
    import concourse.mybir as mybir
    n = 0
    for f in nc.m.functions:
        for blk in f.blocks:
            insts = blk.instructions
            out = []
            dirty = False
            for inst in insts:
                si = inst.sync_info
                waits = list(si.on_wait) if (si and si.on_wait) else []
                limit = 1
                if len(waits) > limit:
                    ups = list(si.on_update) if (si and si.on_update) else []
                    up_ids = {u.id for u in ups}
                    # keep own-queue credit waits inline (DMA flow control)
                    waits.sort(key=lambda w: 0 if w.id in up_ids else 1)
                    keep, move = waits[:limit], waits[limit:]
                    for w in move:
                        n += 1
                        car = mybir.InstEventSemaphore(
                            name="WSPLIT-%d" % n, ins=[], outs=[])
                        car.engine = inst.engine
                        car.sync_info = mybir.SyncInfo(on_wait=[w],
                                                       on_update=[])
                        out.append(car)
                    inst.sync_info = mybir.SyncInfo(on_wait=keep,
                                                   on_update=ups)
                    dirty = True
                out.append(inst)
            if dirty:
                blk.instructions = out
    return n


def _prep_weights(inputs):
    """Host-side weight folding. Returns dict of np arrays + flags."""
    f32 = np.float32
    g1 = np.asarray(inputs["g1"], f32)
    beta1 = np.asarray(inputs["beta1"], f32)
    g2 = np.asarray(inputs["g2"], f32)
    beta2 = np.asarray(inputs["beta2"], f32)
    Wq = np.asarray(inputs["Wq"], f32)
    Wk = np.asarray(inputs["Wk"], f32)
    Wv = np.asarray(inputs["Wv"], f32)
    Wo = np.asarray(inputs["Wo"], f32)
    W1 = np.asarray(inputs["W1"], f32)
    W2 = np.asarray(inputs["W2"], f32)
    scale = np.float32(1.0 / np.sqrt(D))

    rwq = (Wq.T * g1[:, None] * scale).astype(BF)
    rwk = (Wk.T * g1[:, None]).astype(BF)
    rwv = (Wv.T * g1[:, None]).astype(BF)
    # rwo2[par*64+d, tau, e] = Wo.T[(2*tau+par)*64+d, e]
    rwo2 = np.ascontiguousarray(
        Wo.T.reshape(4, 2, 64, E).transpose(1, 2, 0, 3).reshape(128, 4, E)
    ).astype(BF)
    rw1 = (W1.T * g2[:, None] * FP8_S1).astype(FP8)
    w2t = (W2.T * FP8_S2).astype(FP8)

    c2q = ((Wq @ beta1 + np.asarray(inputs["bq"], f32)) * scale).astype(f32)
    c2k = (Wk @ beta1 + np.asarray(inputs["bk"], f32)).astype(f32)
    c2v = (Wv @ beta1 + np.asarray(inputs["bv"], f32)).astype(f32)
    bo = np.asarray(inputs["bo"], f32)
    c2f = (W1 @ beta2 + np.asarray(inputs["b1"], f32)).astype(f32)
    b2 = np.asarray(inputs["b2"], f32)

    mask = np.zeros((128, 128), f32)
    for i in range(16):
        for gg in range(8):
            for hh in range(8):
                mask[gg * 16 + i, hh * 16 + i] = 1.0

    return dict(
        rwq=rwq, rwk=rwk, rwv=rwv, rwo2=rwo2, rw1=rw1, w2t=w2t,
        mask=mask.astype(BF),
        c2q=c2q, c2k=c2k, c2v=c2v, bo=bo.astype(BF), c2f=c2f, b2=b2,
        has_qkv_bias=bool(np.any(c2q) or np.any(c2k) or np.any(c2v)),
        has_bo=bool(np.any(bo)), has_c2f=bool(np.any(c2f)),
        has_b2=bool(np.any(b2)),
    )


def kernel(**inputs):
    from concourse.bass_utils import run_bass_kernel_spmd

    x = np.asarray(inputs["x"], np.float32)
    n = x.shape[0]
    npc = n // N_CORES
    w = _prep_weights(inputs)

    nc = build_nc(npc, has_qkv_bias=w["has_qkv_bias"], has_bo=w["has_bo"],
                  has_c2f=w["has_c2f"], has_b2=w["has_b2"])

    shared = dict(rwq=w["rwq"], rwk=w["rwk"], rwv=w["rwv"], rwo2=w["rwo2"],
                  rw1=w["rw1"], w2t=w["w2t"], mask=w["mask"],
                  c2q=w["c2q"], c2k=w["c2k"], c2v=w["c2v"], bo=w["bo"],
                  c2f=w["c2f"], b2=w["b2"])
    in_maps = []
    for core in range(N_CORES):
        m = dict(shared)
        m["x"] = np.ascontiguousarray(x[core * npc:(core + 1) * npc])
        in_maps.append(m)

    res = run_bass_kernel_spmd(nc, in_maps, list(range(N_CORES)))
    out = np.concatenate([np.asarray(res.results[c]["out"])
                          for c in range(N_CORES)], axis=0)
    return out.astype(np.float32)


# revision 34
# speedup vs baseline: 1.2733x; 1.0929x over previous
"""Trainium2 Bass kernel for EnhancedGraphTransformerLayer.

Layer: LN1 -> QKV proj -> per-node 8x8 head attention -> O proj -> residual
       -> LN2 -> FFN(512->2048->512, relu) -> residual.

Strategy (per NeuronCore, data-parallel over nodes, 8 cores):
- All big matmuls in bf16 on the PE (fp32 accumulate in PSUM).
- QKV projections are chunked over 4 node-groups (512-col matmuls,
  weights stationary) from a feature-transposed z chunk.
- Per-node 8-head attention uses "sub-group" packing: for each 16-node
  sub-group s, a (64, 128) slice layout q`T[d, h*16+j] lets one matmul
  compute all 128x128 head-pair scores; a block mask zeroes cross-node
  terms after exp, and an appended ones-column of V yields softmax
  denominators inside the AV matmul.
- The packing shuffles (q/k/v extraction, and the inverse extraction
  that feeds a K=128 O-projection) are merged into 2 DMAs per tensor
  and spread across the SP/Activation HWDGE queues + Pool SWDGE so they
  run concurrently with PE work instead of serializing on gpsimd.
- Attention is software-pipelined across groups (stage1: scores/AV and
  the repack DMA; stage2: O-proj + LN2) so the PE never waits on the
  inverse-extraction DMA latency.
- LayerNorm stats via bn_stats/bn_aggr in natural layout; gamma/beta
  folded into weights/biases on the host.
"""

import os

import numpy as np
import ml_dtypes
from contextlib import ExitStack

E = 512
H = 8
D = 64
F = 2048
EPS = 1e-5
FP8_S1 = 64.0
FP8_S2 = 64.0
FP8_SQ = 512.0
FP8_SK = 64.0
N_NODES = 65536
N_CORES = 8
BF = ml_dtypes.bfloat16
FP8 = ml_dtypes.float8_e4m3


def build_nc(npc, has_qkv_bias=False, has_bo=False, has_c2f=False,
             has_b2=False):
    import concourse.bass as bass
    import concourse.mybir as mybir

    f32 = mybir.dt.float32
    bf16 = mybir.dt.bfloat16
    fp8 = mybir.dt.float8e4

    nc = bass.Bass()
    ins = dict(
        x=nc.dram_tensor("x", (npc, E), f32, kind="ExternalInput").ap(),
        rwq=nc.dram_tensor("rwq", (E, E), fp8, kind="ExternalInput").ap(),
        rwk=nc.dram_tensor("rwk", (E, E), fp8, kind="ExternalInput").ap(),
        rwv=nc.dram_tensor("rwv", (E, E), fp8, kind="ExternalInput").ap(),
        rwo2=nc.dram_tensor("rwo2", (128, 4, E), bf16, kind="ExternalInput").ap(),
        rw1=nc.dram_tensor("rw1", (E, F), fp8, kind="ExternalInput").ap(),
        w2t=nc.dram_tensor("w2t", (F, E), fp8, kind="ExternalInput").ap(),
        mask=nc.dram_tensor("mask", (128, 128), bf16, kind="ExternalInput").ap(),
        c2q=nc.dram_tensor("c2q", (E,), f32, kind="ExternalInput").ap(),
        c2k=nc.dram_tensor("c2k", (E,), f32, kind="ExternalInput").ap(),
        c2v=nc.dram_tensor("c2v", (E,), f32, kind="ExternalInput").ap(),
        bo=nc.dram_tensor("bo", (E,), bf16, kind="ExternalInput").ap(),
        c2f=nc.dram_tensor("c2f", (F,), f32, kind="ExternalInput").ap(),
        b2=nc.dram_tensor("b2", (E,), f32, kind="ExternalInput").ap(),
    )
    out_ap = nc.dram_tensor("out", (npc, E), f32, kind="ExternalOutput").ap()
    build_body(nc, ins, out_ap, npc, has_qkv_bias=has_qkv_bias,
               has_bo=has_bo, has_c2f=has_c2f, has_b2=has_b2)
    return nc


def build_body(nc, ins, out_d, npc, has_qkv_bias=False, has_bo=False,
               has_c2f=False, has_b2=False):
    import concourse.bass as bass
    import concourse.mybir as mybir
    from concourse.tile import TileContext
    from concourse.masks import make_identity

    f32 = mybir.dt.float32
    bf16 = mybir.dt.bfloat16
    fp8 = mybir.dt.float8e4
    PM = mybir.MatmulPerfMode
    AL = mybir.AluOpType
    AF = mybir.ActivationFunctionType

    n_groups = npc // 128
    gpc = 4 if n_groups % 4 == 0 else 1  # groups per chunk
    n_chunks = n_groups // gpc

    x_d = ins["x"]
    rwq_d, rwk_d, rwv_d, rwo2_d = ins["rwq"], ins["rwk"], ins["rwv"], ins["rwo2"]
    rw1_d, w2t_d, mask_d = ins["rw1"], ins["w2t"], ins["mask"]
    c2q_d, c2k_d, c2v_d = ins["c2q"], ins["c2k"], ins["c2v"]
    bo_d, c2f_d, b2_d = ins["bo"], ins["c2f"], ins["b2"]

    with TileContext(nc) as tc, ExitStack() as ctx:
        wpool = ctx.enter_context(tc.tile_pool(name="w", bufs=1))
        pool = ctx.enter_context(tc.tile_pool(name="act", bufs=1))
        psum = ctx.enter_context(tc.tile_pool(name="ps", bufs=1, space="PSUM"))

        # ---- constants / weights ----
        rwq_sb = wpool.tile([128, 4, E], fp8, tag="rwq")
        rwk_sb = wpool.tile([128, 4, E], fp8, tag="rwk")
        rwv_sb = wpool.tile([128, 4, E], fp8, tag="rwv")
        rwo2_sb = wpool.tile([128, 4, E], bf16, tag="rwo2")
        nc.sync.dma_start(out=rwq_sb, in_=rwq_d.rearrange("(t p) e -> p t e", p=128))
        nc.sync.dma_start(out=rwk_sb, in_=rwk_d.rearrange("(t p) e -> p t e", p=128))
        nc.sync.dma_start(out=rwv_sb, in_=rwv_d.rearrange("(t p) e -> p t e", p=128))
        nc.sync.dma_start(out=rwo2_sb, in_=rwo2_d)
        rw1_sb = wpool.tile([128, 4, F], fp8, tag="rw1")
        nc.sync.dma_start(out=rw1_sb, in_=rw1_d.rearrange("(t p) f -> p t f", p=128))
        w2t_sb = wpool.tile([128, 16, E], fp8, tag="w2t")
        nc.sync.dma_start(out=w2t_sb, in_=w2t_d.rearrange("(t p) e -> p t e", p=128))
        mask_sb = wpool.tile([128, 128], bf16, tag="mask")
        nc.sync.dma_start(out=mask_sb, in_=mask_d)
        ident64 = wpool.tile([64, 64], bf16, tag="id64")
        make_identity(nc, ident64)
        ident128 = wpool.tile([128, 128], bf16, tag="id128")
        make_identity(nc, ident128)
        eps_sb = wpool.tile([128, 1], f32, tag="eps")
        nc.vector.memset(eps_sb, EPS)
        if has_qkv_bias:
            c2q_sb = wpool.tile([128, 4], f32, tag="c2q")
            c2k_sb = wpool.tile([128, 4], f32, tag="c2k")
            c2v_sb = wpool.tile([128, 4], f32, tag="c2v")
            nc.sync.dma_start(out=c2q_sb, in_=c2q_d.rearrange("(t p) -> p t", p=128))
            nc.sync.dma_start(out=c2k_sb, in_=c2k_d.rearrange("(t p) -> p t", p=128))
            nc.sync.dma_start(out=c2v_sb, in_=c2v_d.rearrange("(t p) -> p t", p=128))
        if has_bo:
            ones1_sb = wpool.tile([1, 128], bf16, tag="ones1")
            nc.vector.memset(ones1_sb, 1.0)
            bo_sb = wpool.tile([1, E], bf16, tag="bo")
            nc.sync.dma_start(out=bo_sb, in_=bo_d.rearrange("e -> 1 e"))
        if has_c2f:
            c2f_sb = wpool.tile([128, 16], f32, tag="c2f")
            nc.sync.dma_start(out=c2f_sb, in_=c2f_d.rearrange("(t p) -> p t", p=128))
        if has_b2:
            b2_sb = wpool.tile([128, 4], f32, tag="b2")
            nc.sync.dma_start(out=b2_sb, in_=b2_d.rearrange("(t p) -> p t", p=128))

        def bbox2(tile_ap):
            """2-element AP covering the tile's full byte range (fence)."""
            fs = 1
            for st, ct in tile_ap.ap[1:]:
                fs = max(fs, st * ct)
            return bass.AP(tensor=tile_ap.tensor, offset=tile_ap.offset,
                           ap=[tile_ap.ap[0], [fs - 1, 2]])

        def bcast8x64(small):
            """(128, 8) scalar AP broadcast to (128, 8, 64) via stride-0."""
            return bass.AP(tensor=small.tensor, offset=small.offset,
                           ap=[small.ap[0], [1, 8], [0, 64]])

        def headhalf_dst(tile_ap, par):
            """[64, 8(h), 128(n)] tile viewed as the parity-par head half:
            [64, tau(stride 256), n(128)] at column offset 128*par, i.e.
            head h=2*tau+par blocks."""
            return bass.AP(tensor=tile_ap.tensor,
                           offset=tile_ap.offset + 128 * par,
                           ap=[tile_ap.ap[0], [256, 4], [1, 128]])

        def sgrp(tile_ap, s):
            """[64, 8(h), 128(n)] tile sliced to sub-group s: [64 part,
            h(stride 128), j(16)] -> 128 slots streamed in (h, j) order."""
            return bass.AP(tensor=tile_ap.tensor,
                           offset=tile_ap.offset + 16 * s,
                           ap=[tile_ap.ap[0], [128, 8], [1, 16]])

        def ln_stats(x_sb, mvs, gi, tagp):
            """bn stats into mvs[:, gi, :] (mean, var)."""
            stat = pool.tile([128, 6], f32, tag=tagp + "stat", bufs=2, name=tagp + "stat")
            nc.vector.bn_stats(out=stat, in_=x_sb)
            nc.vector.bn_aggr(out=mvs[:, gi, :], in_=stat)

        def ln_sqrt_batch(mvs, tagp):
            """One Sqrt for the whole chunk: rs4[:, gi] = 1/sqrt(var_gi+eps).
            Batching keeps Exp<->Sqrt act-table switches to 2 per chunk."""
            rs4 = pool.tile([128, gpc], f32, tag=tagp + "rs4", bufs=2,
                            name=tagp + "rs4")
            nc.scalar.activation(out=rs4, in_=mvs[:, :, 1], func=AF.Sqrt,
                                 bias=eps_sb, scale=1.0)
            nc.vector.reciprocal(out=rs4, in_=rs4)
            return rs4

        def ln_norm(x_sb, mvs, rs4, gi, tagp):
            zb = pool.tile([128, E], bf16, tag=tagp + "zb", bufs=2, name=tagp + "zb")
            nc.vector.tensor_scalar(out=zb, in0=x_sb, scalar1=mvs[:, gi, 0:1],
                                    scalar2=rs4[:, gi:gi + 1], op0=AL.subtract,
                                    op1=AL.mult)
            return zb

        def prep(c):
            """Loads, LN1, z-transpose, chunked QKV, packing extractions."""
            P = {"x_tiles": [], "x2_tiles": []}
            zbT_chunk = pool.tile([128, 4, gpc, 128], fp8, tag="zchunk",
                                  bufs=2, name="zchunk")
            P["zbT_chunk"] = zbT_chunk
            mvs1 = pool.tile([128, gpc, 2], f32, tag="mvs1", bufs=2, name="mvs1")
            for gi in range(gpc):
                g = c * gpc + gi
                x_sb = pool.tile([128, E], f32, tag="x", bufs=gpc + 2, name="x_sb")
                nc.vector.memset(bbox2(x_sb), 0.0)
                nc.sync.dma_start(out=x_sb, in_=x_d[g * 128:(g + 1) * 128, :])
                P["x_tiles"].append(x_sb)
                ln_stats(x_sb, mvs1, gi, "ln1")
            rs41 = ln_sqrt_batch(mvs1, "ln1")
            for gi in range(gpc):
                zb = ln_norm(P["x_tiles"][gi], mvs1, rs41, gi, "ln1")
                zbT_ps = psum.tile([128, 4, 128], bf16, tag="A", bufs=4, name="zbT_ps")
                for tau in range(4):
                    nc.tensor.transpose(zbT_ps[:, tau, :],
                                        zb[:, 128 * tau:128 * (tau + 1)],
                                        ident128[:, :])
                nc.scalar.activation(out=zbT_chunk[:, :, gi, :], in_=zbT_ps,
                                     func=AF.Copy)

            # ---- QKV projections, chunked (512-col matmuls) ----
            qTb = pool.tile([128, 4, gpc, 128], bf16, tag="qTb", bufs=2, name="qTb")
            kTb = pool.tile([128, 4, gpc, 128], bf16, tag="kTb", bufs=2, name="kTb")
            vTb = pool.tile([128, 4, gpc, 128], bf16, tag="vTb", bufs=2, name="vTb")
            for tau in range(4):
                qT_ps = psum.tile([128, gpc * 128], f32, tag="A", bufs=4, name="qT_ps")
                kT_ps = psum.tile([128, gpc * 128], f32, tag="A", bufs=4, name="kT_ps")
                vT_ps = psum.tile([128, gpc * 128], f32, tag="A", bufs=4, name="vT_ps")
                for u in range(2):
                    st = u == 0
                    sp = u == 1
                    es = slice(2 * u, 2 * u + 2)
                    nc.tensor.matmul(qT_ps,
                                     rwq_sb[:, es, 128 * tau:128 * (tau + 1)],
                                     zbT_chunk[:, es, :, :], start=st, stop=sp,
                                     perf_mode=PM.DoubleRow)
                    nc.tensor.matmul(kT_ps,
                                     rwk_sb[:, es, 128 * tau:128 * (tau + 1)],
                                     zbT_chunk[:, es, :, :], start=st, stop=sp,
                                     perf_mode=PM.DoubleRow)
                    nc.tensor.matmul(vT_ps,
                                     rwv_sb[:, es, 128 * tau:128 * (tau + 1)],
                                     zbT_chunk[:, es, :, :], start=st, stop=sp,
                                     perf_mode=PM.DoubleRow)
                qv = qT_ps.rearrange("p (g n) -> p g n", g=gpc)
                kv = kT_ps.rearrange("p (g n) -> p g n", g=gpc)
                vv = vT_ps.rearrange("p (g n) -> p g n", g=gpc)
                if has_qkv_bias:
                    nc.scalar.activation(out=qTb[:, tau, :, :], in_=qv,
                                         func=AF.Identity, bias=c2q_sb[:, tau:tau + 1],
                                         scale=1.0 / FP8_SQ)
                    nc.scalar.activation(out=kTb[:, tau, :, :], in_=kv,
                                         func=AF.Identity, bias=c2k_sb[:, tau:tau + 1],
                                         scale=1.0 / FP8_SK)
                    nc.vector.tensor_scalar(out=vTb[:, tau, :, :], in0=vv,
                                            scalar1=1.0 / FP8_SK,
                                            scalar2=c2v_sb[:, tau:tau + 1],
                                            op0=AL.mult, op1=AL.add)
                else:
                    nc.scalar.activation(out=qTb[:, tau, :, :], in_=qv,
                                         func=AF.Copy, scale=1.0 / FP8_SQ)
                    nc.scalar.activation(out=kTb[:, tau, :, :], in_=kv,
                                         func=AF.Copy, scale=1.0 / FP8_SK)
                    nc.vector.tensor_scalar(out=vTb[:, tau, :, :], in0=vv,
                                            scalar1=1.0 / FP8_SK, scalar2=0.0,
                                            op0=AL.mult, op1=AL.add)

            # ---- packing extractions: per tensor per group, 2 partition-half
            #      DMAs into [d, h, n] layout (SP / Act / Pool queues), then an
            #      on-chip shuffle copy into the [d, s, h*16+j] matmul layout ----
            qxs, kxs, vxs = [], [], []
            for gi in range(gpc):
                q2 = pool.tile([64, 8, 128], bf16, tag="q2", bufs=2, name="q2")
                k2 = pool.tile([64, 8, 128], bf16, tag="k2", bufs=2, name="k2")
                v2 = pool.tile([64, 8, 128], bf16, tag="v2", bufs=2, name="v2")
                nc.scalar.activation(out=bbox2(q2), in_=bbox2(q2), func=AF.Copy)
                nc.vector.memset(bbox2(k2), 0.0)
                nc.vector.memset(bbox2(v2), 0.0)
                for par in range(2):
                    p0 = par * 64
                    nc.gpsimd.dma_start(out=headhalf_dst(v2, par),
                                        in_=vTb[p0:p0 + 64, :, gi, :],
                                        single_packet=True)
                    nc.sync.dma_start(out=headhalf_dst(q2, par),
                                      in_=qTb[p0:p0 + 64, :, gi, :])
                    nc.scalar.dma_start(out=headhalf_dst(k2, par),
                                        in_=kTb[p0:p0 + 64, :, gi, :])
                qx = pool.tile([64, 8, 128], bf16, tag="qx", bufs=2, name="qx")
                kx = pool.tile([64, 8, 128], bf16, tag="kx", bufs=2, name="kx")
                vx = pool.tile([64, 8, 128], bf16, tag="vx", bufs=2, name="vx")
                nc.scalar.copy(
                    out=vx.rearrange("d s (h j) -> d s h j", h=8),
                    in_=v2.rearrange("d h (s j) -> d s h j", s=8))
                nc.vector.tensor_copy(
                    out=qx.rearrange("d s (h j) -> d s h j", h=8),
                    in_=q2.rearrange("d h (s j) -> d s h j", s=8))
                nc.vector.tensor_copy(
                    out=kx.rearrange("d s (h j) -> d s h j", h=8),
                    in_=k2.rearrange("d h (s j) -> d s h j", s=8))
                qxs.append(qx)
                kxs.append(kx)
                vxs.append(vx)
            P["qxs"], P["kxs"], P["vxs"] = qxs, kxs, vxs
            return P

        def att_gen(c, P):
            """Attention for chunk c, software-pipelined over groups.
            Yields between pieces so FFN(c-1) matmuls can interleave."""
            qxs, kxs, vxs = P["qxs"], P["kxs"], P["vxs"]
            x_tiles, x2_tiles = P["x_tiles"], P["x2_tiles"]
            z2bT_chunk = pool.tile([128, 4, gpc, 128], fp8, tag="z2chunk",
                                   bufs=2, name="z2chunk")
            P["z2bT_chunk"] = z2bT_chunk

            def att_stage1(gi):
                qx, kx, vx = qxs[gi], kxs[gi], vxs[gi]
                # all scores first (exp/mask latency hides behind them)
                s2s = []
                for s0 in range(0, 8, 2):
                    s2_ps = psum.tile([128, 2, 128], f32, tag="A", bufs=4,
                                      name="s2_ps")
                    for si in range(2):
                        s = s0 + si
                        nc.tensor.matmul(s2_ps[:, si, :], kx[:, s, :], qx[:, s, :],
                                         start=True, stop=True)
                    e_sb = pool.tile([128, 2, 128], bf16, tag="esb", bufs=4,
                                     name="e_sb")
                    nc.scalar.activation(out=e_sb, in_=s2_ps, func=AF.Exp)
                    a_sb = pool.tile([128, 2, 128], bf16, tag="asb", bufs=4,
                                     name="a_sb")
                    nc.gpsimd.tensor_tensor(out=a_sb[:, 0, :], in0=e_sb[:, 0, :],
                                            in1=mask_sb, op=AL.mult)
                    nc.vector.tensor_tensor(out=a_sb[:, 1, :], in0=e_sb[:, 1, :],
                                            in1=mask_sb, op=AL.mult)
                    s2s.append(a_sb)
                yield

                # v packing: PE-transpose to [(g,j'), d] + ones column
                vp_ps = psum.tile([128, 8, 64], bf16, tag="A", bufs=4, name="vp_ps")
                for s in range(8):
                    nc.tensor.transpose(vp_ps[:, s, :], vx[:, s, :], ident64[:, :])
                vaug = pool.tile([128, 8, 66], bf16, tag="vaug", bufs=2, name="vaug")
                nc.vector.memset(vaug[:, :, 64:65], 1.0)
                nc.vector.tensor_copy(out=vaug[:, :, 0:64], in_=vp_ps)
                yield

                # AV (+denominator)
                # ([128, 8, 128] keeps each sub-group's slice PSUM-bank aligned)
                outS = psum.tile([128, 8, 128], f32, tag="B", bufs=2, name="outS")
                for s0 in range(0, 8, 2):
                    a_sb = s2s[s0 // 2]
                    for si in range(2):
                        s = s0 + si
                        nc.tensor.matmul(outS[:, s, 0:65], a_sb[:, si, :],
                                         vaug[:, s, 0:65], start=True, stop=True)
                yield

                # normalize + transpose to [d, h, n] layout
                recip = pool.tile([128, 8], f32, tag="recip", bufs=2, name="recip")
                nc.vector.reciprocal(out=recip, in_=outS[:, :, 64])
                ogb = pool.tile([128, 8, 64], bf16, tag="ogb", bufs=2, name="ogb")
                nc.vector.tensor_tensor(out=ogb, in0=outS[:, :, 0:64],
                                        in1=bcast8x64(recip), op=AL.mult)
                p_ps = psum.tile([64, 8, 128], bf16, tag="B", bufs=2, name="p_ps")
                for s in range(8):
                    nc.tensor.transpose(p_ps[:, s, :], ogb[:, s, :], ident128[:, :])
                # shuffle-copy to [d, h, n] while draining PSUM
                p_sb = pool.tile([64, 8, 128], bf16, tag="psb", bufs=2, name="p_sb")
                nc.vector.tensor_copy(
                    out=p_sb.rearrange("d h (s j) -> d s h j", s=8),
                    in_=p_ps.rearrange("d s (h j) -> d s h j", h=8))

                # inverse extraction: op2[(par*64+d), tau, n] <- p_sb[d, 2t+par, n]
                op2 = pool.tile([128, 4, 128], bf16, tag="op2", bufs=2, name="op2")
                nc.scalar.activation(out=bbox2(op2), in_=bbox2(op2), func=AF.Copy)
                for par in range(2):
                    nc.sync.dma_start(out=op2[64 * par:64 * (par + 1), :, :],
                                      in_=headhalf_dst(p_sb, par))
                yield
                return op2

            mvs2 = pool.tile([128, gpc, 2], f32, tag="mvs2", bufs=2, name="mvs2")

            def att_stage2a(gi, op2):
                # O projection, K=128 over 4 tau tiles -> natural [n, e]
                oproj_ps = psum.tile([128, E], f32, tag="A", bufs=4, name="oproj_ps")
                for tau in range(4):
                    nc.tensor.matmul(oproj_ps, op2[:, tau, :], rwo2_sb[:, tau, :],
                                     start=(tau == 0),
                                     stop=(tau == 3 and not has_bo))
                if has_bo:
                    nc.tensor.matmul(oproj_ps, ones1_sb, bo_sb,
                                     start=False, stop=True)

                # residual 1 + LN2 stats
                x2_sb = pool.tile([128, E], f32, tag="x2", bufs=gpc + 2,
                                  name="x2_sb")
                nc.vector.tensor_add(out=x2_sb, in0=x_tiles[gi], in1=oproj_ps)
                x2_tiles.append(x2_sb)
                ln_stats(x2_sb, mvs2, gi, "ln2")

            def att_stage2b(gi, rs42):
                z2b = ln_norm(x2_tiles[gi], mvs2, rs42, gi, "ln2")
                z2bT_ps = psum.tile([128, 4, 128], bf16, tag="A", bufs=4,
                                    name="z2bT_ps")
                for tau in range(4):
                    nc.tensor.transpose(z2bT_ps[:, tau, :],
                                        z2b[:, 128 * tau:128 * (tau + 1)],
                                        ident128[:, :])
                nc.scalar.activation(out=z2bT_chunk[:, :, gi, :], in_=z2bT_ps,
                                     func=AF.Copy)

            ops = [None] * gpc
            for gi in range(gpc):
                ops[gi] = yield from att_stage1(gi)
                if gi > 1:
                    att_stage2a(gi - 2, ops[gi - 2])
                    yield
            att_stage2a(gpc - 2, ops[gpc - 2])
            yield
            att_stage2a(gpc - 1, ops[gpc - 1])
            rs42 = ln_sqrt_batch(mvs2, "ln2")
            for gi in range(gpc):
                att_stage2b(gi, rs42)
                yield

        def ffn_gen(c, P):
            """FFN + output for chunk c. Yields between matmul groups."""
            z2bT_chunk = P["z2bT_chunk"]
            x2_tiles = P["x2_tiles"]
            rT_sb = pool.tile([128, 16, 128 * gpc], fp8, tag="rt", bufs=1, name="rT_sb")
            for ft in range(16):
                u1_ps = psum.tile([128, 128 * gpc], f32, tag="A", bufs=4, name="u1_ps")
                for u in range(2):
                    nc.tensor.matmul(u1_ps,
                                     rw1_sb[:, 2 * u:2 * u + 2, 128 * ft:128 * (ft + 1)],
                                     z2bT_chunk[:, 2 * u:2 * u + 2, :, :],
                                     start=(u == 0), stop=(u == 1),
                                     perf_mode=PM.DoubleRow)
                if has_c2f:
                    nc.scalar.activation(out=rT_sb[:, ft, :], in_=u1_ps,
                                         func=AF.Relu, bias=c2f_sb[:, ft:ft + 1],
                                         scale=1.0 / FP8_S1)
                elif ft % 2 == 0:
                    nc.scalar.activation(out=rT_sb[:, ft, :], in_=u1_ps,
                                         func=AF.Relu, scale=1.0 / FP8_S1)
                else:
                    nc.vector.tensor_scalar(out=rT_sb[:, ft, :], in0=u1_ps,
                                            scalar1=1.0 / FP8_S1, scalar2=0.0,
                                            op0=AL.mult, op1=AL.max)
                if ft % 2 == 1:
                    yield
            u2b_sb = pool.tile([128, 4, 128 * gpc], bf16, tag="u2b", bufs=2, name="u2b_sb")
            for et in range(4):
                u2_ps = psum.tile([128, 128 * gpc], f32, tag="A", bufs=4, name="u2_ps")
                for u in range(8):
                    nc.tensor.matmul(u2_ps,
                                     w2t_sb[:, 2 * u:2 * u + 2, 128 * et:128 * (et + 1)],
                                     rT_sb[:, 2 * u:2 * u + 2, :],
                                     start=(u == 0), stop=(u == 7),
                                     perf_mode=PM.DoubleRow)
                if has_b2:
                    nc.vector.tensor_scalar(out=u2b_sb[:, et, :], in0=u2_ps,
                                            scalar1=1.0 / FP8_S2,
                                            scalar2=b2_sb[:, et:et + 1],
                                            op0=AL.mult, op1=AL.add)
                elif et % 2 == 0:
                    nc.scalar.activation(out=u2b_sb[:, et, :], in_=u2_ps,
                                         func=AF.Copy, scale=1.0 / FP8_S2)
                else:
                    nc.vector.tensor_scalar(out=u2b_sb[:, et, :], in0=u2_ps,
                                            scalar1=1.0 / FP8_S2, scalar2=0.0,
                                            op0=AL.mult, op1=AL.add)
                yield
            u2nat = pool.tile([128, gpc, 4, 128], bf16, tag="u2nat", bufs=2, name="u2nat")
            for gi2 in range(gpc):
                u2n_ps = psum.tile([128, 4, 128], bf16, tag="A", bufs=4, name="u2n_ps")
                for et in range(4):
                    nc.tensor.transpose(u2n_ps[:, et, :],
                                        u2b_sb[:, et, 128 * gi2:128 * (gi2 + 1)],
                                        ident128[:, :])
                if gi2 % 2 == 0:
                    nc.vector.tensor_copy(out=u2nat[:, gi2, :, :], in_=u2n_ps)
                else:
                    nc.scalar.activation(out=u2nat[:, gi2, :, :], in_=u2n_ps,
                                         func=AF.Copy)
                yield
            for gi in range(gpc):
                g = c * gpc + gi
                out_sb = pool.tile([128, E], f32, tag="osb", bufs=3, name="out_sb")
                nc.vector.tensor_add(out=out_sb, in0=x2_tiles[gi],
                                     in1=u2nat[:, gi, :, :])
                nc.gpsimd.dma_start(out=out_d[g * 128:(g + 1) * 128, :], in_=out_sb)

        def drain_interleaved(g1, g2):
            """Round-robin two instruction-emitting generators."""
            gens = [g for g in (g1, g2) if g is not None]
            while gens:
                nxt = []
                for g in gens:
                    try:
                        next(g)
                        nxt.append(g)
                    except StopIteration:
                        pass
                gens = nxt

        # ---- main pipeline: ATT(c) interleaved with FFN(c-1), then PREP(c+1)
        P = [None] * n_chunks
        P[0] = prep(0)
        for c in range(n_chunks):
            drain_interleaved(att_gen(c, P[c]),
                              ffn_gen(c - 1, P[c - 1]) if c > 0 else None)
            if c + 1 < n_chunks:
                P[c + 1] = prep(c + 1)
        drain_interleaved(ffn_gen(n_chunks - 1, P[n_chunks - 1]), None)

    _fix_sync_waits(nc)


_DMA_LIKE = ("InstDMACopy", "InstDmaTransposeAnt", "InstDMATranspose",
             "InstKVWritebackAnt", "InstPagedWritebackAnt")


def _fix_sync_waits(nc):
    """walrus limits inline sync waits to 1 per instruction. Tile can
    emit more. Split the excess into
    standalone InstEventSemaphore wait-carriers inserted immediately before
    the overweight instruction on the same engine - semantically identical
    (the waits still execute right before the instruction, in order)."""
    import concourse.mybir as mybir
    n = 0
    for f in nc.m.functions:
        for blk in f.blocks:
            insts = blk.instructions
            out = []
            dirty = False
            for inst in insts:
                si = inst.sync_info
                waits = list(si.on_wait) if (si and si.on_wait) else []
                limit = 1
                if len(waits) > limit:
                    ups = list(si.on_update) if (si and si.on_update) else []
                    up_ids = {u.id for u in ups}
                    # keep own-queue credit waits inline (DMA flow control)
                    waits.sort(key=lambda w: 0 if w.id in up_ids else 1)
                    keep, move = waits[:limit], waits[limit:]
                    for w in move:
                        n += 1
                        car = mybir.InstEventSemaphore(
                            name="WSPLIT-%d" % n, ins=[], outs=[])
                        car.engine = inst.engine
                        car.sync_info = mybir.SyncInfo(on_wait=[w],
                                                       on_update=[])
                        out.append(car)
                    inst.sync_info = mybir.SyncInfo(on_wait=keep,
                                                   on_update=ups)
                    dirty = True
                out.append(inst)
            if dirty:
                blk.instructions = out
    return n


def _prep_weights(inputs):
    """Host-side weight folding. Returns dict of np arrays + flags."""
    f32 = np.float32
    g1 = np.asarray(inputs["g1"], f32)
    beta1 = np.asarray(inputs["beta1"], f32)
    g2 = np.asarray(inputs["g2"], f32)
    beta2 = np.asarray(inputs["beta2"], f32)
    Wq = np.asarray(inputs["Wq"], f32)
    Wk = np.asarray(inputs["Wk"], f32)
    Wv = np.asarray(inputs["Wv"], f32)
    Wo = np.asarray(inputs["Wo"], f32)
    W1 = np.asarray(inputs["W1"], f32)
    W2 = np.asarray(inputs["W2"], f32)
    scale = np.float32(1.0 / np.sqrt(D))

    rwq = (Wq.T * g1[:, None] * scale * FP8_SQ).astype(FP8)
    rwk = (Wk.T * g1[:, None] * FP8_SK).astype(FP8)
    rwv = (Wv.T * g1[:, None] * FP8_SK).astype(FP8)
    # rwo2[par*64+d, tau, e] = Wo.T[(2*tau+par)*64+d, e]
    rwo2 = np.ascontiguousarray(
        Wo.T.reshape(4, 2, 64, E).transpose(1, 2, 0, 3).reshape(128, 4, E)
    ).astype(BF)
    rw1 = (W1.T * g2[:, None] * FP8_S1).astype(FP8)
    w2t = (W2.T * FP8_S2).astype(FP8)

    c2q = ((Wq @ beta1 + np.asarray(inputs["bq"], f32)) * scale).astype(f32)
    c2k = (Wk @ beta1 + np.asarray(inputs["bk"], f32)).astype(f32)
    c2v = (Wv @ beta1 + np.asarray(inputs["bv"], f32)).astype(f32)
    bo = np.asarray(inputs["bo"], f32)
    c2f = (W1 @ beta2 + np.asarray(inputs["b1"], f32)).astype(f32)
    b2 = np.asarray(inputs["b2"], f32)

    mask = np.zeros((128, 128), f32)
    for i in range(16):
        for gg in range(8):
            for hh in range(8):
                mask[gg * 16 + i, hh * 16 + i] = 1.0

    return dict(
        rwq=rwq, rwk=rwk, rwv=rwv, rwo2=rwo2, rw1=rw1, w2t=w2t,
        mask=mask.astype(BF),
        c2q=c2q, c2k=c2k, c2v=c2v, bo=bo.astype(BF), c2f=c2f, b2=b2,
        has_qkv_bias=bool(np.any(c2q) or np.any(c2k) or np.any(c2v)),
        has_bo=bool(np.any(bo)), has_c2f=bool(np.any(c2f)),
        has_b2=bool(np.any(b2)),
    )


def kernel(**inputs):
    from concourse.bass_utils import run_bass_kernel_spmd

    x = np.asarray(inputs["x"], np.float32)
    n = x.shape[0]
    npc = n // N_CORES
    w = _prep_weights(inputs)

    nc = build_nc(npc, has_qkv_bias=w["has_qkv_bias"], has_bo=w["has_bo"],
                  has_c2f=w["has_c2f"], has_b2=w["has_b2"])

    shared = dict(rwq=w["rwq"], rwk=w["rwk"], rwv=w["rwv"], rwo2=w["rwo2"],
                  rw1=w["rw1"], w2t=w["w2t"], mask=w["mask"],
                  c2q=w["c2q"], c2k=w["c2k"], c2v=w["c2v"], bo=w["bo"],
                  c2f=w["c2f"], b2=w["b2"])
    in_maps = []
    for core in range(N_CORES):
        m = dict(shared)
        m["x"] = np.ascontiguousarray(x[core * npc:(core + 1) * npc])
        in_maps.append(m)

    res = run_bass_kernel_spmd(nc, in_maps, list(range(N_CORES)))
    out = np.concatenate([np.asarray(res.results[c]["out"])
                          for c in range(N_CORES)], axis=0)
    return out.astype(np.float32)
